# revision 4
# baseline (speedup 1.0000x reference)
"""Trainium2 Bass kernel v2 for nn_CMF_Block (cross-modal fusion block).

Reference computation (per batch b):
    q = gconv1x1(rgb, w_q, b_q)   # [c, n]   c=256, n=h*w=4096, groups=4
    k = gconv1x1(ir,  w_k, b_k)
    v = gconv1x1(ir,  w_v, b_v)
    attn = softmax(q^T k * c^-0.5, axis=-1)      # [n, n]
    z = v @ attn^T                                # [c, n]
    y = w4 @ z + b4 ; y = BN(y) ; out = rgb + mish(y)

Sharding: 8 cores = 4 batches x 2 query-halves. Each core gets the full
ir slab [256, 4096] plus its rgb query-half [256, 2048] and produces the
matching disjoint output slice [256, 2048]. No collectives.

v2 design vs baseline (129.2us):
  - z matmul (P@V) in fp8e4 DoubleRow (0.5 cyc/col); P written as fp8 by
    the exp stage, vT staged fp8 with 272-stride r-padding (16B dual-fp8
    alignment). Runs as 2 passes of 2 i-tiles so zps needs only 2 psum
    banks, freeing 6 banks for a triple-buffered score pool.
  - softmax exp split across ACT (AF.Exp) and DVE (Schraudolph exp:
    int8(s*8/ln2 + 56) bitcast fp8e4; ~1.5e-4 end-to-end).
  - k bias dropped (constant per query in scores -> cancels in softmax).
  - mish phase mostly on the idle Pool engine (SBUF-only chain).
  - gconv psum shares the score pool slots, woven into the pair stream.
  - i-group boundary work (znorm, pass-2 z, transposes, phase5) is
    deferred and drained into the next i-group's pair stream so no engine
    queue blocks at boundaries; the last group uses a latency-optimized
    ACT/DVE-only phase5 in 256-col chunks.
  - prologue DMAs issue from 4 queues in parallel, critical chunks first.
"""

import sys

sys.path.insert(0, "/opt/trn_rl_repo")

import numpy as np
import ml_dtypes

import concourse.bass as bass
import concourse.tile as tile
from concourse import bacc
from concourse import mybir
from concourse.bass_utils import run_bass_kernel_spmd
from concourse.masks import make_identity

F32 = mybir.dt.float32
F32R = mybir.dt.float32r
BF16 = mybir.dt.bfloat16
FP8 = mybir.dt.float8e4
I8 = mybir.dt.int8
AF = mybir.ActivationFunctionType
DR = mybir.MatmulPerfMode.DoubleRow
ALU = mybir.AluOpType

BS, C, H, W = 4, 256, 64, 64
N = H * W              # 4096
G, CG = 4, 64
NH = N // 2            # 2048 query positions per core
NCORES = 8
SCALE = C ** -0.5      # 1/16

JT = N // 128          # 32 key tiles
IG = 4                 # i-groups of 512 queries
PAIRS = JT // 2        # 16 j-tile pairs per i-group

A_SCH = float(8.0 / np.log(2.0))   # fp8e4 Schraudolph scale
B_SCH = 56.0                       # fp8e4 exponent bias * 8

# exp engine split: pair indices (mod 16) sent to DVE-Schraudolph.
# Keep the first pairs of each ig on ACT so deferred boundary work on DVE
# isn't stuck behind an exp.
DVE_EXP = {2, 4, 7, 9, 12, 14}


def build_program():
    nc = bacc.Bacc("TRN2", target_bir_lowering=False, debug=False,
                   enable_asserts=False)

    x_rgb = nc.dram_tensor("x_rgb", [C, NH], F32R, kind="ExternalInput").ap()
    x_ir = nc.dram_tensor("x_ir", [C, N], F32R, kind="ExternalInput").ap()
    wq_bd = nc.dram_tensor("wq_bd", [128, 256], F32R, kind="ExternalInput").ap()
    wk_bd = nc.dram_tensor("wk_bd", [128, 256], F32R, kind="ExternalInput").ap()
    wv_r = nc.dram_tensor("wv_r", [128, 512], F32R, kind="ExternalInput").ap()
    w4t = nc.dram_tensor("w4t", [128, 512], BF16, kind="ExternalInput").ap()
    bq = nc.dram_tensor("bq", [128, 2], F32, kind="ExternalInput").ap()
    bk = nc.dram_tensor("bk", [128, 2], F32, kind="ExternalInput").ap()
    b4 = nc.dram_tensor("b4", [128, 2], F32, kind="ExternalInput").ap()
    out = nc.dram_tensor("out", [C, NH], F32, kind="ExternalOutput").ap()

    with tile.TileContext(nc) as tc:
        with tc.tile_pool(name="persist", bufs=1) as persist:
            qsg = [persist.tile([128, 2, 512], FP8, tag=f"qsg{g}",
                                name=f"qsg{g}") for g in range(IG)]
            ksh = [persist.tile([128, 2, 2048], FP8, tag=f"ksh{h}",
                                name=f"ksh{h}") for h in range(2)]
            # vT2 packs j-tile pairs for DoubleRow: [p, jj, r, c] =
            # v[(16h + 2jj + r)*128 + p, c]; col 256 = ones (softmax denom),
            # cols 257:272 pad for the 16B-aligned r-stride.
            vT2 = [persist.tile([128, 8, 2, 272], FP8, tag=f"vT2{h}",
                                name=f"vT2{h}") for h in range(2)]
            zsg = [[persist.tile([128, 512], BF16, tag=f"zsg{ch}_{g}",
                                 name=f"zsg{ch}_{g}") for g in range(IG)]
                   for ch in range(2)]
            rgbf = [persist.tile([128, NH], F32R, tag=f"rgbf{ch}",
                                 name=f"rgbf{ch}") for ch in range(2)]
            irf = [[persist.tile([128, 2048], F32R, tag=f"irf{ch}_{h}",
                                 name=f"irf{ch}_{h}") for h in range(2)]
                   for ch in range(2)]
            wq_sb = persist.tile([128, 2, 128], F32R, tag="wq_sb", name="wq_sb")
            wk_sb = persist.tile([128, 2, 128], F32R, tag="wk_sb", name="wk_sb")
            wv_sb = persist.tile([128, 2, 256], F32R, tag="wv_sb", name="wv_sb")
            w4_sb = persist.tile([128, 2, 2, 128], BF16, tag="w4_sb", name="w4_sb")
            bq_sb = persist.tile([128, 2], F32, tag="bq_sb", name="bq_sb")
            b4_sb = persist.tile([128, 2], F32, tag="b4_sb", name="b4_sb")
            ident = persist.tile([128, 128], BF16, tag="ident", name="ident")

            # ---- prologue DMAs: critical chunks first (HWDGE serializes
            # issues globally, so minimize count and front-load the head) ----
            nc.sync.dma_start(wk_sb[:], wk_bd)
            nc.sync.dma_start(irf[0][0][:, 0:1024], x_ir[0:128, 0:1024])
            nc.sync.dma_start(irf[1][0][:, 0:1024], x_ir[128:256, 0:1024])
            nc.sync.dma_start(wq_sb[:], wq_bd)
            nc.sync.dma_start(bq_sb[:], bq)
            nc.sync.dma_start(rgbf[0][:, 0:512], x_rgb[0:128, 0:512])
            nc.sync.dma_start(rgbf[1][:, 0:512], x_rgb[128:256, 0:512])
            nc.sync.dma_start(wv_sb[:], wv_r)
            nc.sync.dma_start(irf[0][0][:, 1024:2048], x_ir[0:128, 1024:2048])
            nc.sync.dma_start(irf[1][0][:, 1024:2048], x_ir[128:256, 1024:2048])
            nc.sync.dma_start(irf[0][1][:], x_ir[0:128, 2048:4096])
            nc.sync.dma_start(irf[1][1][:], x_ir[128:256, 2048:4096])
            nc.sync.dma_start(rgbf[0][:, 512:2048], x_rgb[0:128, 512:2048])
            nc.sync.dma_start(rgbf[1][:, 512:2048], x_rgb[128:256, 512:2048])
            nc.sync.dma_start(w4_sb[:], w4t)
            nc.sync.dma_start(b4_sb[:], b4)
            make_identity(nc, ident[:])
            for h in range(2):
                nc.gpsimd.memset(vT2[h][:, :, :, 256:257], 1.0)

            with (
                tc.tile_pool(name="spool", bufs=3, space="PSUM") as spool,
                tc.tile_pool(name="zpool", bufs=2, space="PSUM") as zpool,
                tc.tile_pool(name="pexp", bufs=20) as pexp,
                tc.tile_pool(name="znorm", bufs=8) as znorm,
                tc.tile_pool(name="fin", bufs=2) as fin,
            ):
                # ---- woven gconv slot tasks (share s-pool with pairs) ----
                mv_alt = [0]

                def mover(dst, src, bias=None):
                    mv_alt[0] ^= 1
                    if bias is None:
                        if mv_alt[0]:
                            nc.vector.tensor_copy(dst, src)
                        else:
                            nc.scalar.copy(dst, src)
                    else:
                        if mv_alt[0]:
                            nc.vector.tensor_scalar_add(dst, src, bias)
                        else:
                            nc.scalar.activation(dst, src, AF.Identity,
                                                 bias=bias)

                def kslot(h, ch, half):
                    ps = spool.tile([128, 1024], F32, tag="s", name="kps")
                    for q4 in range(2):
                        csl = slice(half * 1024 + q4 * 512,
                                    half * 1024 + (q4 + 1) * 512)
                        nc.tensor.matmul(ps[:, q4 * 512:(q4 + 1) * 512],
                                         wk_sb[:, ch], irf[ch][h][:, csl],
                                         start=True, stop=True)
                    csl = slice(half * 1024, (half + 1) * 1024)
                    mover(ksh[h][:, ch, csl], ps[:])

                def qslot(g):
                    gsl = slice(g * 512, (g + 1) * 512)
                    ps = spool.tile([128, 1024], F32, tag="s", name="qps")
                    for ch in range(2):
                        nc.tensor.matmul(ps[:, ch * 512:(ch + 1) * 512],
                                         wq_sb[:, ch], rgbf[ch][:, gsl],
                                         start=True, stop=True)
                    for ch in range(2):
                        mover(qsg[g][:, ch, :], ps[:, ch * 512:(ch + 1) * 512],
                              bias=bq_sb[:, ch:ch + 1])

                def vslot(h, q):
                    # 4 j-tiles (j = 4q .. 4q+3 within half h) -> vT2
                    ps = spool.tile([128, 1024], F32, tag="s", name="vps")
                    for jl in range(4):
                        j = 4 * q + jl
                        jsl = slice(j * 128, (j + 1) * 128)
                        psl = slice(jl * 256, (jl + 1) * 256)
                        for ch in range(2):
                            nc.tensor.matmul(ps[:, psl], irf[ch][h][:, jsl],
                                             wv_sb[:, ch],
                                             start=(ch == 0), stop=(ch == 1))
                    dst = vT2[h][:, 2 * q:2 * q + 2, :, 0:256]
                    mover(dst, ps[:])

                weave = {
                    0: [lambda: kslot(0, 0, 0), lambda: kslot(0, 1, 0),
                        lambda: qslot(0)],
                    1: [lambda: vslot(0, 0)],
                    2: [lambda: vslot(0, 1)],
                    3: [lambda: kslot(0, 0, 1)],
                    4: [lambda: kslot(0, 1, 1)],
                    5: [lambda: vslot(0, 2), lambda: kslot(1, 0, 0)],
                    6: [lambda: vslot(0, 3), lambda: kslot(1, 1, 0)],
                    7: [lambda: kslot(1, 0, 1)],
                    8: [lambda: kslot(1, 1, 1)],
                    9: [lambda: vslot(1, 0)],
                    10: [lambda: vslot(1, 1)],
                    11: [lambda: vslot(1, 2)],
                    12: [lambda: vslot(1, 3)],
                    13: [lambda: qslot(1)],
                    18: [lambda: qslot(2)],
                    34: [lambda: qslot(3)],
                }

                def phase5_mm(g, oh):
                    ps = zpool.tile([128, 512], F32, tag="z", name="yps")
                    for ch in range(2):
                        nc.tensor.matmul(ps[:], w4_sb[:, ch, oh],
                                         zsg[ch][g][:],
                                         start=(ch == 0), stop=(ch == 1))
                    return ps

                def phase5_chain(g, oh, ps, last):
                    bias = b4_sb[:, oh:oh + 1]
                    # mish(y) = y*tanh(softplus(y)); with u = e^y:
                    # tanh(softplus(y)) = 1 - 2/((u+1)^2+1)
                    chunks = 2 if last else 1
                    cw = 512 // chunks
                    for cix in range(chunks):
                        cs = slice(cix * cw, (cix + 1) * cw)
                        psc = ps[:, cs]
                        u = fin.tile([128, cw], F32, tag=f"u{cix}", name="u")
                        nc.scalar.activation(u[:], psc, AF.Exp, bias=bias)
                        w2 = fin.tile([128, cw], F32, tag=f"w2{cix}", name="w2")
                        nc.scalar.activation(w2[:], u[:], AF.Square, bias=1.0)
                        dd = fin.tile([128, cw], F32, tag=f"dd{cix}", name="dd")
                        t = fin.tile([128, cw], F32, tag=f"t{cix}", name="t")
                        rr = fin.tile([128, cw], F32, tag=f"rr{cix}", name="rr")
                        if last:
                            nc.vector.tensor_scalar_add(dd[:], w2[:], 1.0)
                            nc.vector.reciprocal(rr[:], dd[:])
                            nc.scalar.activation(t[:], rr[:], AF.Identity,
                                                 bias=1.0, scale=-2.0)
                        else:
                            nc.gpsimd.tensor_scalar_add(dd[:], w2[:], 1.0)
                            nc.vector.reciprocal(rr[:], dd[:])
                            nc.gpsimd.tensor_scalar(t[:], rr[:], -2.0, 1.0,
                                                    ALU.mult, ALU.add)
                        m = fin.tile([128, cw], F32, tag=f"m{cix}", name="m")
                        nc.vector.scalar_tensor_tensor(m[:], psc, bias, t[:],
                                                       ALU.add, ALU.mult)
                        o = fin.tile([128, cw], F32, tag=f"o{cix}", name="o")
                        rslice = rgbf[oh][:, g * 512 + cix * cw:
                                          g * 512 + (cix + 1) * cw]
                        nc.gpsimd.tensor_add(o[:], m[:],
                                             rslice.bitcast(F32))
                        nc.sync.dma_start(
                            out[oh * 128:(oh + 1) * 128,
                                g * 512 + cix * cw:g * 512 + (cix + 1) * cw],
                            o[:])

                deferq = []

                def drain(k):
                    for _ in range(k):
                        if deferq:
                            deferq.pop(0)()

                for ig in range(IG):
                    lastig = ig == IG - 1
                    inline2 = lastig    # last ig: single-pass z, t2/t3 in
                    pend = []           # a long-lived s-pool slot
                    pts = []
                    zps1 = []          # [t0, t1] psum tiles, lazy
                    zps2 = []          # [t2, t3]

                    def flush1(pair, zps1=zps1, zps2=zps2, lastig=inline2):
                        pt, pr = pair
                        if not zps1:
                            zps1.extend(zpool.tile([128, 257], F32, tag="z",
                                                   name=f"zp1_{t}")
                                        for t in range(2))
                        if lastig and not zps2:
                            # last ig: inline single-pass for t2/t3 too, in a
                            # long-lived s-pool slot (bank-aligned regions)
                            z2 = spool.tile([128, 2, 512], F32, tag="s",
                                            name="zp2sp")
                            zps2.extend([(z2[:, tt, 0:257],
                                          z2[:, tt, 256:257],
                                          z2[:, tt, 0:256])
                                         for tt in range(2)])
                        h, jj = pr // 8, pr % 8
                        rhs = vT2[h][:, jj, :, 0:257]
                        for t in range(2):
                            nc.tensor.matmul(
                                zps1[t][:], pt[:, :, t * 128:(t + 1) * 128],
                                rhs, perf_mode=DR,
                                start=(pr == 0), stop=(pr == PAIRS - 1))
                        if lastig:
                            for t in range(2, 4):
                                nc.tensor.matmul(
                                    zps2[t - 2][0],
                                    pt[:, :, t * 128:(t + 1) * 128],
                                    rhs, perf_mode=DR,
                                    start=(pr == 0), stop=(pr == PAIRS - 1))

                    for pr in range(PAIRS):
                        gp = ig * PAIRS + pr
                        for task in weave.get(gp, []):
                            task()
                        ps = spool.tile([128, 2, 512], F32, tag="s", name="sT")
                        for hh in range(2):
                            jt = 2 * pr + hh
                            jsl = slice((jt % 16) * 128, (jt % 16 + 1) * 128)
                            nc.tensor.matmul(ps[:, hh], ksh[jt // 16][:, :, jsl],
                                             qsg[ig][:], perf_mode=DR,
                                             start=True, stop=True)
                        if len(pend) > 2:
                            flush1(pend.pop(0))
                        drain(4)
                        pt = pexp.tile([128, 2, 512], FP8, tag="pt", name="pt")
                        if pr in DVE_EXP:
                            nc.vector.tensor_scalar(pt[:].bitcast(I8), ps[:],
                                                    A_SCH, B_SCH,
                                                    ALU.mult, ALU.add)
                        else:
                            nc.scalar.activation(pt[:], ps[:], AF.Exp)
                        pend.append((pt, pr))
                        pts.append(pt)

                    # ---- boundary work, deferred into next ig's stream ----
                    def boundary(ig=ig, pend=list(pend), pts=list(pts),
                                 zps1=zps1, zps2=zps2):
                        last = ig == IG - 1
                        rinvs = [None] * 4
                        zns = [None] * 4

                        def ftails():
                            for pair in pend:
                                flush1(pair, zps1=zps1)

                        def zn_one(t, src_den, src_dat):
                            rinvs[t] = znorm.tile([128, 1], F32,
                                                  tag=f"ri{t}", name="ri")
                            nc.vector.reciprocal(rinvs[t][:], src_den)
                            zns[t] = znorm.tile([128, 256], BF16,
                                                tag=f"zn{t}", name="zn")
                            if last and t % 2 == 0:
                                nc.scalar.activation(zns[t][:], src_dat,
                                                     AF.Identity,
                                                     scale=rinvs[t][:, 0:1])
                            else:
                                nc.vector.tensor_scalar_mul(
                                    zns[t][:], src_dat, rinvs[t][:])

                        def norm01():
                            for t in range(2):
                                zn_one(t, zps1[t][:, 256:257],
                                       zps1[t][:, 0:256])

                        def pass2(t):
                            if last:
                                return   # t2/t3 accumulated inline
                            if not zps2:
                                for tt in range(2):
                                    zt = zpool.tile([128, 257], F32, tag="z",
                                                    name=f"zp2_{tt}")
                                    zps2.append((zt[:], zt[:, 256:257],
                                                 zt[:, 0:256]))
                            for pr2, pt2 in enumerate(pts):
                                h, jj = pr2 // 8, pr2 % 8
                                nc.tensor.matmul(
                                    zps2[t - 2][0],
                                    pt2[:, :, t * 128:(t + 1) * 128],
                                    vT2[h][:, jj, :, 0:257], perf_mode=DR,
                                    start=(pr2 == 0), stop=(pr2 == PAIRS - 1))

                        def norm23():
                            for t in range(2, 4):
                                zn_one(t, zps2[t - 2][1], zps2[t - 2][2])

                        def transp(tp2):
                            # i-tiles 2*tp2, 2*tp2+1 for both chunks
                            for ch in range(2):
                                tp = zpool.tile([128, 256], BF16, tag="z",
                                                name="tp")
                                for k in range(2):
                                    t = 2 * tp2 + k
                                    nc.tensor.transpose(
                                        tp[:, k * 128:(k + 1) * 128],
                                        zns[t][:, ch * 128:(ch + 1) * 128],
                                        ident[:])
                                dst = zsg[ch][ig][:,
                                                  tp2 * 256:(tp2 + 1) * 256]
                                if last and ch == 0:
                                    nc.scalar.copy(dst, tp[:])
                                else:
                                    nc.vector.tensor_copy(dst, tp[:])

                        ph5ps = [None, None]

                        def mms():
                            ph5ps[0] = phase5_mm(ig, 0)
                            ph5ps[1] = phase5_mm(ig, 1)

                        if last:
                            return [ftails,
                                    lambda: (norm01(), norm23()),
                                    lambda: transp(0), lambda: transp(1),
                                    mms,
                                    lambda: phase5_chain(ig, 0, ph5ps[0],
                                                         last),
                                    lambda: phase5_chain(ig, 1, ph5ps[1],
                                                         last)]
                        return [ftails, norm01,
                                lambda: transp(0),
                                lambda: pass2(2), lambda: pass2(3),
                                norm23,
                                lambda: transp(1),
                                mms,
                                lambda: phase5_chain(ig, 0, ph5ps[0], last),
                                lambda: phase5_chain(ig, 1, ph5ps[1], last)]

                    deferq.extend(boundary())

                # drain the tail (last ig boundary work)
                while deferq:
                    drain(1)

    nc.finalize()
    return nc


def _blockdiag_T(w, g0, g1):
    """lhsT chunk: [[w[g0].T, 0], [0, w[g1].T]] as [128, 128]."""
    m = np.zeros((128, 128), dtype=np.float64)
    m[:64, :64] = w[g0].T
    m[64:, 64:] = w[g1].T
    return m


def prep_inputs(rgb, ir, w_q, b_q, w_k, b_k, w_v, b_v, w4, b4,
                gamma, beta, rmean, rvar):
    """Host-side prep: fold scale/BN/b_v, pack block-diagonal weights."""
    f64 = np.float64
    w_q, b_q = f64(np.asarray(w_q)), f64(np.asarray(b_q))
    w_k = f64(np.asarray(w_k))
    w_v, b_v = f64(np.asarray(w_v)), f64(np.asarray(b_v))
    w4, b4 = f64(np.asarray(w4)), f64(np.asarray(b4))
    gamma, beta = f64(np.asarray(gamma)), f64(np.asarray(beta))
    rmean, rvar = f64(np.asarray(rmean)), f64(np.asarray(rvar))

    inv = gamma / np.sqrt(rvar + 1e-5)
    w4f = w4 * inv[:, None]                      # BN folded into w4
    b4f = b4 * inv + beta - rmean * inv + w4f @ b_v   # b_v folded

    f32 = np.float32
    bf16 = ml_dtypes.bfloat16
    hs = np.sqrt(SCALE)  # split attention scale between q and k for fp8 range
    wq_bd = np.concatenate([_blockdiag_T(w_q * hs, 0, 1),
                            _blockdiag_T(w_q * hs, 2, 3)], axis=1).astype(f32)
    wk_bd = np.concatenate([_blockdiag_T(w_k * hs, 0, 1),
                            _blockdiag_T(w_k * hs, 2, 3)], axis=1).astype(f32)
    wv_r = np.zeros((128, 512), dtype=np.float64)
    wv_r[:, 0:128] = _blockdiag_T(w_v, 0, 1)
    wv_r[:, 384:512] = _blockdiag_T(w_v, 2, 3)
    wv_r = wv_r.astype(f32)
    w4t = np.zeros((128, 512), dtype=np.float64)
    for ch in range(2):
        for oh in range(2):
            w4t[:, ch * 256 + oh * 128:ch * 256 + (oh + 1) * 128] = \
                w4f[oh * 128:(oh + 1) * 128, ch * 128:(ch + 1) * 128].T
    w4t = w4t.astype(bf16)

    def cols(v):
        return np.stack([v[:128], v[128:]], axis=1).astype(np.float32)

    bq_c = cols(b_q * hs)
    bk_c = np.zeros((128, 2), dtype=np.float32)  # k bias cancels in softmax
    b4_c = cols(b4f)

    rgb_f = np.ascontiguousarray(np.asarray(rgb), dtype=np.float32)
    ir_f = np.ascontiguousarray(np.asarray(ir), dtype=np.float32)

    weights = dict(wq_bd=wq_bd, wk_bd=wk_bd, wv_r=wv_r, w4t=w4t,
                   bq=bq_c, bk=bk_c, b4=b4_c)
    in_maps = []
    for core in range(NCORES):
        b, half = divmod(core, 2)
        x_rgb = np.ascontiguousarray(
            rgb_f[b].reshape(C, N)[:, half * NH:(half + 1) * NH])
        x_ir = np.ascontiguousarray(ir_f[b].reshape(C, N))
        in_maps.append(dict(x_rgb=x_rgb, x_ir=x_ir, **weights))
    return in_maps


_PROGRAM = None


def _get_program():
    global _PROGRAM
    if _PROGRAM is None:
        _PROGRAM = build_program()
    return _PROGRAM


def run(inputs, trace=False, **kw):
    """Run on 8 cores; returns (full_output, BassKernelResults)."""
    nc = _get_program()
    in_maps = prep_inputs(**inputs)
    res = run_bass_kernel_spmd(nc, in_maps, list(range(NCORES)),
                               trace=trace, **kw)
    full = np.zeros((BS, C, H, W), dtype=np.float32)
    for core in range(NCORES):
        b, half = divmod(core, 2)
        full[b].reshape(C, N)[:, half * NH:(half + 1) * NH] = \
            res.results[core]["out"]
    return full, res


def kernel(**inputs) -> np.ndarray:
    out, _ = run(inputs)
    return out


# revision 5
# speedup vs baseline: 1.0285x; 1.0285x over previous
"""Trainium2 Bass kernel v2 for nn_CMF_Block (cross-modal fusion block).

Reference computation (per batch b):
    q = gconv1x1(rgb, w_q, b_q)   # [c, n]   c=256, n=h*w=4096, groups=4
    k = gconv1x1(ir,  w_k, b_k)
    v = gconv1x1(ir,  w_v, b_v)
    attn = softmax(q^T k * c^-0.5, axis=-1)      # [n, n]
    z = v @ attn^T                                # [c, n]
    y = w4 @ z + b4 ; y = BN(y) ; out = rgb + mish(y)

Sharding: 8 cores = 4 batches x 2 query-halves. Each core gets the full
ir slab [256, 4096] plus its rgb query-half [256, 2048] and produces the
matching disjoint output slice [256, 2048]. No collectives.

v2 design vs baseline (129.2us):
  - z matmul (P@V) in fp8e4 DoubleRow (0.5 cyc/col); P written as fp8 by
    the exp stage, vT staged fp8 with 272-stride r-padding (16B dual-fp8
    alignment). Runs as 2 passes of 2 i-tiles so zps needs only 2 psum
    banks, freeing 6 banks for a triple-buffered score pool.
  - softmax exp split across ACT (AF.Exp) and DVE (Schraudolph exp:
    int8(s*8/ln2 + 56) bitcast fp8e4; ~1.5e-4 end-to-end).
  - k bias dropped (constant per query in scores -> cancels in softmax).
  - mish phase mostly on the idle Pool engine (SBUF-only chain).
  - gconv psum shares the score pool slots, woven into the pair stream.
  - i-group boundary work (znorm, pass-2 z, transposes, phase5) is
    deferred and drained into the next i-group's pair stream so no engine
    queue blocks at boundaries; the last group uses a latency-optimized
    ACT/DVE-only phase5 in 256-col chunks.
  - prologue DMAs issue from 4 queues in parallel, critical chunks first.
"""

import sys

sys.path.insert(0, "/opt/trn_rl_repo")

import numpy as np
import ml_dtypes

import concourse.bass as bass
import concourse.tile as tile
from concourse import bacc
from concourse import mybir
from concourse.bass_utils import run_bass_kernel_spmd
from concourse.masks import make_identity

F32 = mybir.dt.float32
F32R = mybir.dt.float32r
BF16 = mybir.dt.bfloat16
FP8 = mybir.dt.float8e4
I8 = mybir.dt.int8
AF = mybir.ActivationFunctionType
DR = mybir.MatmulPerfMode.DoubleRow
ALU = mybir.AluOpType

BS, C, H, W = 4, 256, 64, 64
N = H * W              # 4096
G, CG = 4, 64
NH = N // 2            # 2048 query positions per core
NCORES = 8
SCALE = C ** -0.5      # 1/16

JT = N // 128          # 32 key tiles
IG = 4                 # i-groups of 512 queries
PAIRS = JT // 2        # 16 j-tile pairs per i-group

A_SCH = float(8.0 / np.log(2.0))   # fp8e4 Schraudolph scale
B_SCH = 56.0                       # fp8e4 exponent bias * 8

# exp engine split: pair indices (mod 16) sent to DVE-Schraudolph.
# Keep the first pairs of each ig on ACT so deferred boundary work on DVE
# isn't stuck behind an exp.
DVE_EXP = {4, 6, 8, 10, 12, 15}


def build_program():
    nc = bacc.Bacc("TRN2", target_bir_lowering=False, debug=False,
                   enable_asserts=False)

    x_rgb = nc.dram_tensor("x_rgb", [C, NH], F32R, kind="ExternalInput").ap()
    x_ir = nc.dram_tensor("x_ir", [C, N], F32R, kind="ExternalInput").ap()
    wq_bd = nc.dram_tensor("wq_bd", [128, 256], F32R, kind="ExternalInput").ap()
    wk_bd = nc.dram_tensor("wk_bd", [128, 256], F32R, kind="ExternalInput").ap()
    wv_r = nc.dram_tensor("wv_r", [128, 512], F32R, kind="ExternalInput").ap()
    w4t = nc.dram_tensor("w4t", [128, 512], BF16, kind="ExternalInput").ap()
    bq = nc.dram_tensor("bq", [128, 2], F32, kind="ExternalInput").ap()
    bk = nc.dram_tensor("bk", [128, 2], F32, kind="ExternalInput").ap()
    b4 = nc.dram_tensor("b4", [128, 2], F32, kind="ExternalInput").ap()
    out = nc.dram_tensor("out", [C, NH], F32, kind="ExternalOutput").ap()

    with tile.TileContext(nc) as tc:
        with tc.tile_pool(name="persist", bufs=1) as persist:
            qsg = [persist.tile([128, 2, 512], FP8, tag=f"qsg{g}",
                                name=f"qsg{g}") for g in range(IG)]
            ksh = [persist.tile([128, 2, 2048], FP8, tag=f"ksh{h}",
                                name=f"ksh{h}") for h in range(2)]
            # vT2 packs j-tile pairs for DoubleRow: [p, jj, r, c] =
            # v[(16h + 2jj + r)*128 + p, c]; col 256 = ones (softmax denom),
            # cols 257:272 pad for the 16B-aligned r-stride.
            vT2 = [persist.tile([128, 8, 2, 272], FP8, tag=f"vT2{h}",
                                name=f"vT2{h}") for h in range(2)]
            zsg = [[persist.tile([128, 512], BF16, tag=f"zsg{ch}_{g}",
                                 name=f"zsg{ch}_{g}") for g in range(IG)]
                   for ch in range(2)]
            rgbf = [persist.tile([128, NH], F32R, tag=f"rgbf{ch}",
                                 name=f"rgbf{ch}") for ch in range(2)]
            irf = [[persist.tile([128, 2048], F32R, tag=f"irf{ch}_{h}",
                                 name=f"irf{ch}_{h}") for h in range(2)]
                   for ch in range(2)]
            wq_sb = persist.tile([128, 2, 128], F32R, tag="wq_sb", name="wq_sb")
            wk_sb = persist.tile([128, 2, 128], F32R, tag="wk_sb", name="wk_sb")
            wv_sb = persist.tile([128, 2, 256], F32R, tag="wv_sb", name="wv_sb")
            w4_sb = persist.tile([128, 2, 2, 128], BF16, tag="w4_sb", name="w4_sb")
            bq_sb = persist.tile([128, 2], F32, tag="bq_sb", name="bq_sb")
            b4_sb = persist.tile([128, 2], F32, tag="b4_sb", name="b4_sb")
            ident = persist.tile([128, 128], BF16, tag="ident", name="ident")

            # ---- prologue DMAs: critical chunks first (HWDGE serializes
            # issues globally, so minimize count and front-load the head) ----
            nc.sync.dma_start(wk_sb[:], wk_bd)
            nc.sync.dma_start(irf[0][0][:, 0:1024], x_ir[0:128, 0:1024])
            nc.sync.dma_start(irf[1][0][:, 0:1024], x_ir[128:256, 0:1024])
            nc.sync.dma_start(wq_sb[:], wq_bd)
            nc.sync.dma_start(bq_sb[:], bq)
            nc.sync.dma_start(rgbf[0][:, 0:512], x_rgb[0:128, 0:512])
            nc.sync.dma_start(rgbf[1][:, 0:512], x_rgb[128:256, 0:512])
            nc.sync.dma_start(wv_sb[:], wv_r)
            nc.sync.dma_start(irf[0][0][:, 1024:2048], x_ir[0:128, 1024:2048])
            nc.sync.dma_start(irf[1][0][:, 1024:2048], x_ir[128:256, 1024:2048])
            nc.sync.dma_start(irf[0][1][:], x_ir[0:128, 2048:4096])
            nc.sync.dma_start(irf[1][1][:], x_ir[128:256, 2048:4096])
            nc.sync.dma_start(rgbf[0][:, 512:2048], x_rgb[0:128, 512:2048])
            nc.sync.dma_start(rgbf[1][:, 512:2048], x_rgb[128:256, 512:2048])
            nc.sync.dma_start(w4_sb[:], w4t)
            nc.sync.dma_start(b4_sb[:], b4)
            make_identity(nc, ident[:])
            for h in range(2):
                nc.gpsimd.memset(vT2[h][:, :, :, 256:257], 1.0)

            with (
                tc.tile_pool(name="spool", bufs=3, space="PSUM") as spool,
                tc.tile_pool(name="zpool", bufs=2, space="PSUM") as zpool,
                tc.tile_pool(name="pexp", bufs=20) as pexp,
                tc.tile_pool(name="znorm", bufs=8) as znorm,
                tc.tile_pool(name="fin", bufs=2) as fin,
            ):
                # ---- woven gconv slot tasks (share s-pool with pairs) ----
                mv_alt = [0]

                def mover(dst, src, bias=None):
                    mv_alt[0] ^= 1
                    if bias is None:
                        if mv_alt[0]:
                            nc.vector.tensor_copy(dst, src)
                        else:
                            nc.scalar.copy(dst, src)
                    else:
                        if mv_alt[0]:
                            nc.vector.tensor_scalar_add(dst, src, bias)
                        else:
                            nc.scalar.activation(dst, src, AF.Identity,
                                                 bias=bias)

                def kslot(h, ch, half, fast=False):
                    ps = spool.tile([128, 1024], F32, tag="s", name="kps")
                    for q4 in range(2):
                        csl = slice(half * 1024 + q4 * 512,
                                    half * 1024 + (q4 + 1) * 512)
                        nc.tensor.matmul(ps[:, q4 * 512:(q4 + 1) * 512],
                                         wk_sb[:, ch], irf[ch][h][:, csl],
                                         start=True, stop=True)
                        if fast:
                            mover(ksh[h][:, ch, csl],
                                  ps[:, q4 * 512:(q4 + 1) * 512])
                    if not fast:
                        csl = slice(half * 1024, (half + 1) * 1024)
                        mover(ksh[h][:, ch, csl], ps[:])

                def qslot(g):
                    gsl = slice(g * 512, (g + 1) * 512)
                    ps = spool.tile([128, 1024], F32, tag="s", name="qps")
                    for ch in range(2):
                        nc.tensor.matmul(ps[:, ch * 512:(ch + 1) * 512],
                                         wq_sb[:, ch], rgbf[ch][:, gsl],
                                         start=True, stop=True)
                    for ch in range(2):
                        mover(qsg[g][:, ch, :], ps[:, ch * 512:(ch + 1) * 512],
                              bias=bq_sb[:, ch:ch + 1])

                def vslot(h, q):
                    # 4 j-tiles (j = 4q .. 4q+3 within half h) -> vT2
                    ps = spool.tile([128, 1024], F32, tag="s", name="vps")
                    for jl in range(4):
                        j = 4 * q + jl
                        jsl = slice(j * 128, (j + 1) * 128)
                        psl = slice(jl * 256, (jl + 1) * 256)
                        for ch in range(2):
                            nc.tensor.matmul(ps[:, psl], irf[ch][h][:, jsl],
                                             wv_sb[:, ch],
                                             start=(ch == 0), stop=(ch == 1))
                    dst = vT2[h][:, 2 * q:2 * q + 2, :, 0:256]
                    mover(dst, ps[:])

                weave = {
                    0: [lambda: kslot(0, 0, 0), lambda: kslot(0, 1, 0),
                        lambda: qslot(0)],
                    1: [lambda: vslot(0, 0)],
                    2: [lambda: vslot(0, 1)],
                    3: [lambda: kslot(0, 0, 1)],
                    4: [lambda: kslot(0, 1, 1)],
                    5: [lambda: vslot(0, 2), lambda: kslot(1, 0, 0)],
                    6: [lambda: vslot(0, 3), lambda: kslot(1, 1, 0)],
                    7: [lambda: kslot(1, 0, 1)],
                    8: [lambda: kslot(1, 1, 1)],
                    9: [lambda: vslot(1, 0)],
                    10: [lambda: vslot(1, 1)],
                    11: [lambda: vslot(1, 2)],
                    12: [lambda: vslot(1, 3)],
                    13: [lambda: qslot(1)],
                    18: [lambda: qslot(2)],
                    34: [lambda: qslot(3)],
                }

                def phase5_mm(g, oh):
                    ps = zpool.tile([128, 512], F32, tag="z", name="yps")
                    for ch in range(2):
                        nc.tensor.matmul(ps[:], w4_sb[:, ch, oh],
                                         zsg[ch][g][:],
                                         start=(ch == 0), stop=(ch == 1))
                    return ps

                def phase5_chain(g, oh, ps, last):
                    bias = b4_sb[:, oh:oh + 1]
                    # mish(y) = y*tanh(softplus(y)); with u = e^y:
                    # tanh(softplus(y)) = 1 - 2/((u+1)^2+1)
                    chunks = 1
                    cw = 512 // chunks
                    for cix in range(chunks):
                        cs = slice(cix * cw, (cix + 1) * cw)
                        psc = ps[:, cs]
                        u = fin.tile([128, cw], F32, tag=f"u{cix}", name="u")
                        nc.scalar.activation(u[:], psc, AF.Exp, bias=bias)
                        w2 = fin.tile([128, cw], F32, tag=f"w2{cix}", name="w2")
                        nc.scalar.activation(w2[:], u[:], AF.Square, bias=1.0)
                        dd = fin.tile([128, cw], F32, tag=f"dd{cix}", name="dd")
                        t = fin.tile([128, cw], F32, tag=f"t{cix}", name="t")
                        rr = fin.tile([128, cw], F32, tag=f"rr{cix}", name="rr")
                        if last:
                            nc.vector.tensor_scalar_add(dd[:], w2[:], 1.0)
                            nc.vector.reciprocal(rr[:], dd[:])
                            nc.scalar.activation(t[:], rr[:], AF.Identity,
                                                 bias=1.0, scale=-2.0)
                        else:
                            nc.gpsimd.tensor_scalar_add(dd[:], w2[:], 1.0)
                            nc.vector.reciprocal(rr[:], dd[:])
                            nc.gpsimd.tensor_scalar(t[:], rr[:], -2.0, 1.0,
                                                    ALU.mult, ALU.add)
                        m = fin.tile([128, cw], F32, tag=f"m{cix}", name="m")
                        nc.vector.scalar_tensor_tensor(m[:], psc, bias, t[:],
                                                       ALU.add, ALU.mult)
                        o = fin.tile([128, cw], F32, tag=f"o{cix}", name="o")
                        rslice = rgbf[oh][:, g * 512 + cix * cw:
                                          g * 512 + (cix + 1) * cw]
                        if last:
                            nc.vector.tensor_add(o[:], m[:],
                                                 rslice.bitcast(F32))
                        else:
                            nc.gpsimd.tensor_add(o[:], m[:],
                                                 rslice.bitcast(F32))
                        nc.sync.dma_start(
                            out[oh * 128:(oh + 1) * 128,
                                g * 512 + cix * cw:g * 512 + (cix + 1) * cw],
                            o[:])

                deferq = []

                def drain(k):
                    for _ in range(k):
                        if deferq:
                            deferq.pop(0)()

                for ig in range(IG):
                    lastig = ig == IG - 1
                    inline2 = lastig    # last ig: single-pass z, t2/t3 in
                    pend = []           # a long-lived s-pool slot
                    pts = []
                    zps1 = []          # [t0, t1] psum tiles, lazy
                    zps2 = []          # [t2, t3]

                    def flush1(pair, zps1=zps1, zps2=zps2, lastig=inline2):
                        pt, pr = pair
                        if not zps1:
                            zps1.extend(zpool.tile([128, 257], F32, tag="z",
                                                   name=f"zp1_{t}")
                                        for t in range(2))
                        if lastig and not zps2:
                            # last ig: inline single-pass for t2/t3 too, in a
                            # long-lived s-pool slot (bank-aligned regions)
                            z2 = spool.tile([128, 2, 512], F32, tag="s",
                                            name="zp2sp")
                            zps2.extend([(z2[:, tt, 0:257],
                                          z2[:, tt, 256:257],
                                          z2[:, tt, 0:256])
                                         for tt in range(2)])
                        h, jj = pr // 8, pr % 8
                        rhs = vT2[h][:, jj, :, 0:257]
                        for t in range(2):
                            nc.tensor.matmul(
                                zps1[t][:], pt[:, :, t * 128:(t + 1) * 128],
                                rhs, perf_mode=DR,
                                start=(pr == 0), stop=(pr == PAIRS - 1))
                        if lastig:
                            for t in range(2, 4):
                                nc.tensor.matmul(
                                    zps2[t - 2][0],
                                    pt[:, :, t * 128:(t + 1) * 128],
                                    rhs, perf_mode=DR,
                                    start=(pr == 0), stop=(pr == PAIRS - 1))

                    for pr in range(PAIRS):
                        gp = ig * PAIRS + pr
                        for task in weave.get(gp, []):
                            task()
                        ps = spool.tile([128, 2, 512], F32, tag="s", name="sT")
                        for hh in range(2):
                            jt = 2 * pr + hh
                            jsl = slice((jt % 16) * 128, (jt % 16 + 1) * 128)
                            nc.tensor.matmul(ps[:, hh], ksh[jt // 16][:, :, jsl],
                                             qsg[ig][:], perf_mode=DR,
                                             start=True, stop=True)
                        if len(pend) > 2:
                            flush1(pend.pop(0))
                        drain(4)
                        pt = pexp.tile([128, 2, 512], FP8, tag="pt", name="pt")
                        if pr in DVE_EXP:
                            nc.vector.tensor_scalar(pt[:].bitcast(I8), ps[:],
                                                    A_SCH, B_SCH,
                                                    ALU.mult, ALU.add)
                        else:
                            nc.scalar.activation(pt[:], ps[:], AF.Exp)
                        pend.append((pt, pr))
                        pts.append(pt)

                    # ---- boundary work, deferred into next ig's stream ----
                    def boundary(ig=ig, pend=list(pend), pts=list(pts),
                                 zps1=zps1, zps2=zps2):
                        last = ig == IG - 1
                        rinvs = [None] * 4
                        zns = [None] * 4

                        def ftails():
                            for pair in pend:
                                flush1(pair, zps1=zps1)

                        def zn_one(t, src_den, src_dat):
                            rinvs[t] = znorm.tile([128, 1], F32,
                                                  tag=f"ri{t}", name="ri")
                            nc.vector.reciprocal(rinvs[t][:], src_den)
                            zns[t] = znorm.tile([128, 256], BF16,
                                                tag=f"zn{t}", name="zn")
                            if last and t % 2 == 0:
                                nc.scalar.activation(zns[t][:], src_dat,
                                                     AF.Identity,
                                                     scale=rinvs[t][:, 0:1])
                            else:
                                nc.vector.tensor_scalar_mul(
                                    zns[t][:], src_dat, rinvs[t][:])

                        def norm01():
                            for t in range(2):
                                zn_one(t, zps1[t][:, 256:257],
                                       zps1[t][:, 0:256])

                        def pass2(t):
                            if last:
                                return   # t2/t3 accumulated inline
                            if not zps2:
                                for tt in range(2):
                                    zt = zpool.tile([128, 257], F32, tag="z",
                                                    name=f"zp2_{tt}")
                                    zps2.append((zt[:], zt[:, 256:257],
                                                 zt[:, 0:256]))
                            for pr2, pt2 in enumerate(pts):
                                h, jj = pr2 // 8, pr2 % 8
                                nc.tensor.matmul(
                                    zps2[t - 2][0],
                                    pt2[:, :, t * 128:(t + 1) * 128],
                                    vT2[h][:, jj, :, 0:257], perf_mode=DR,
                                    start=(pr2 == 0), stop=(pr2 == PAIRS - 1))

                        def norm23():
                            for t in range(2, 4):
                                zn_one(t, zps2[t - 2][1], zps2[t - 2][2])

                        def transp(tp2):
                            # i-tiles 2*tp2, 2*tp2+1 for both chunks
                            for ch in range(2):
                                tp = zpool.tile([128, 256], BF16, tag="z",
                                                name="tp")
                                for k in range(2):
                                    t = 2 * tp2 + k
                                    nc.tensor.transpose(
                                        tp[:, k * 128:(k + 1) * 128],
                                        zns[t][:, ch * 128:(ch + 1) * 128],
                                        ident[:])
                                dst = zsg[ch][ig][:,
                                                  tp2 * 256:(tp2 + 1) * 256]
                                if last and ch == 0:
                                    nc.scalar.copy(dst, tp[:])
                                else:
                                    nc.vector.tensor_copy(dst, tp[:])

                        ph5ps = [None, None]

                        def mms():
                            ph5ps[0] = phase5_mm(ig, 0)
                            ph5ps[1] = phase5_mm(ig, 1)

                        if last:
                            return [ftails,
                                    lambda: (norm01(), norm23()),
                                    lambda: transp(0), lambda: transp(1),
                                    mms,
                                    lambda: phase5_chain(ig, 0, ph5ps[0],
                                                         last),
                                    lambda: phase5_chain(ig, 1, ph5ps[1],
                                                         last)]
                        return [ftails, norm01,
                                lambda: transp(0),
                                lambda: pass2(2), lambda: pass2(3),
                                norm23,
                                lambda: transp(1),
                                mms,
                                lambda: phase5_chain(ig, 0, ph5ps[0], last),
                                lambda: phase5_chain(ig, 1, ph5ps[1], last)]

                    deferq.extend(boundary())

                # drain the tail (last ig boundary work)
                while deferq:
                    drain(1)

    nc.finalize()
    return nc


def _blockdiag_T(w, g0, g1):
    """lhsT chunk: [[w[g0].T, 0], [0, w[g1].T]] as [128, 128]."""
    m = np.zeros((128, 128), dtype=np.float64)
    m[:64, :64] = w[g0].T
    m[64:, 64:] = w[g1].T
    return m


def prep_inputs(rgb, ir, w_q, b_q, w_k, b_k, w_v, b_v, w4, b4,
                gamma, beta, rmean, rvar):
    """Host-side prep: fold scale/BN/b_v, pack block-diagonal weights."""
    f64 = np.float64
    w_q, b_q = f64(np.asarray(w_q)), f64(np.asarray(b_q))
    w_k = f64(np.asarray(w_k))
    w_v, b_v = f64(np.asarray(w_v)), f64(np.asarray(b_v))
    w4, b4 = f64(np.asarray(w4)), f64(np.asarray(b4))
    gamma, beta = f64(np.asarray(gamma)), f64(np.asarray(beta))
    rmean, rvar = f64(np.asarray(rmean)), f64(np.asarray(rvar))

    inv = gamma / np.sqrt(rvar + 1e-5)
    w4f = w4 * inv[:, None]                      # BN folded into w4
    b4f = b4 * inv + beta - rmean * inv + w4f @ b_v   # b_v folded

    f32 = np.float32
    bf16 = ml_dtypes.bfloat16
    hs = np.sqrt(SCALE)  # split attention scale between q and k for fp8 range
    wq_bd = np.concatenate([_blockdiag_T(w_q * hs, 0, 1),
                            _blockdiag_T(w_q * hs, 2, 3)], axis=1).astype(f32)
    wk_bd = np.concatenate([_blockdiag_T(w_k * hs, 0, 1),
                            _blockdiag_T(w_k * hs, 2, 3)], axis=1).astype(f32)
    wv_r = np.zeros((128, 512), dtype=np.float64)
    wv_r[:, 0:128] = _blockdiag_T(w_v, 0, 1)
    wv_r[:, 384:512] = _blockdiag_T(w_v, 2, 3)
    wv_r = wv_r.astype(f32)
    w4t = np.zeros((128, 512), dtype=np.float64)
    for ch in range(2):
        for oh in range(2):
            w4t[:, ch * 256 + oh * 128:ch * 256 + (oh + 1) * 128] = \
                w4f[oh * 128:(oh + 1) * 128, ch * 128:(ch + 1) * 128].T
    w4t = w4t.astype(bf16)

    def cols(v):
        return np.stack([v[:128], v[128:]], axis=1).astype(np.float32)

    bq_c = cols(b_q * hs)
    bk_c = np.zeros((128, 2), dtype=np.float32)  # k bias cancels in softmax
    b4_c = cols(b4f)

    rgb_f = np.ascontiguousarray(np.asarray(rgb), dtype=np.float32)
    ir_f = np.ascontiguousarray(np.asarray(ir), dtype=np.float32)

    weights = dict(wq_bd=wq_bd, wk_bd=wk_bd, wv_r=wv_r, w4t=w4t,
                   bq=bq_c, bk=bk_c, b4=b4_c)
    in_maps = []
    for core in range(NCORES):
        b, half = divmod(core, 2)
        x_rgb = np.ascontiguousarray(
            rgb_f[b].reshape(C, N)[:, half * NH:(half + 1) * NH])
        x_ir = np.ascontiguousarray(ir_f[b].reshape(C, N))
        in_maps.append(dict(x_rgb=x_rgb, x_ir=x_ir, **weights))
    return in_maps


_PROGRAM = None


def _get_program():
    global _PROGRAM
    if _PROGRAM is None:
        _PROGRAM = build_program()
    return _PROGRAM


def run(inputs, trace=False, **kw):
    """Run on 8 cores; returns (full_output, BassKernelResults)."""
    nc = _get_program()
    in_maps = prep_inputs(**inputs)
    res = run_bass_kernel_spmd(nc, in_maps, list(range(NCORES)),
                               trace=trace, **kw)
    full = np.zeros((BS, C, H, W), dtype=np.float32)
    for core in range(NCORES):
        b, half = divmod(core, 2)
        full[b].reshape(C, N)[:, half * NH:(half + 1) * NH] = \
            res.results[core]["out"]
    return full, res


def kernel(**inputs) -> np.ndarray:
    out, _ = run(inputs)
    return out


# revision 6
# speedup vs baseline: 1.0299x; 1.0013x over previous
"""Trainium2 Bass kernel v2 for nn_CMF_Block (cross-modal fusion block).

Reference computation (per batch b):
    q = gconv1x1(rgb, w_q, b_q)   # [c, n]   c=256, n=h*w=4096, groups=4
    k = gconv1x1(ir,  w_k, b_k)
    v = gconv1x1(ir,  w_v, b_v)
    attn = softmax(q^T k * c^-0.5, axis=-1)      # [n, n]
    z = v @ attn^T                                # [c, n]
    y = w4 @ z + b4 ; y = BN(y) ; out = rgb + mish(y)

Sharding: 8 cores = 4 batches x 2 query-halves. Each core gets the full
ir slab [256, 4096] plus its rgb query-half [256, 2048] and produces the
matching disjoint output slice [256, 2048]. No collectives.

v2 design vs baseline (129.2us):
  - z matmul (P@V) in fp8e4 DoubleRow (0.5 cyc/col); P written as fp8 by
    the exp stage, vT staged fp8 with 272-stride r-padding (16B dual-fp8
    alignment). Runs as 2 passes of 2 i-tiles so zps needs only 2 psum
    banks, freeing 6 banks for a triple-buffered score pool.
  - softmax exp split across ACT (AF.Exp) and DVE (Schraudolph exp:
    int8(s*8/ln2 + 56) bitcast fp8e4; ~1.5e-4 end-to-end).
  - k bias dropped (constant per query in scores -> cancels in softmax).
  - mish phase mostly on the idle Pool engine (SBUF-only chain).
  - gconv psum shares the score pool slots, woven into the pair stream.
  - i-group boundary work (znorm, pass-2 z, transposes, phase5) is
    deferred and drained into the next i-group's pair stream so no engine
    queue blocks at boundaries; the last group uses a latency-optimized
    ACT/DVE-only phase5 in 256-col chunks.
  - prologue DMAs issue from 4 queues in parallel, critical chunks first.
"""

import sys

sys.path.insert(0, "/opt/trn_rl_repo")

import numpy as np
import ml_dtypes

import concourse.bass as bass
import concourse.tile as tile
from concourse import bacc
from concourse import mybir
from concourse.bass_utils import run_bass_kernel_spmd
from concourse.masks import make_identity

F32 = mybir.dt.float32
F32R = mybir.dt.float32r
BF16 = mybir.dt.bfloat16
FP8 = mybir.dt.float8e4
I8 = mybir.dt.int8
AF = mybir.ActivationFunctionType
DR = mybir.MatmulPerfMode.DoubleRow
ALU = mybir.AluOpType

BS, C, H, W = 4, 256, 64, 64
N = H * W              # 4096
G, CG = 4, 64
NH = N // 2            # 2048 query positions per core
NCORES = 8
SCALE = C ** -0.5      # 1/16

JT = N // 128          # 32 key tiles
IG = 4                 # i-groups of 512 queries
PAIRS = JT // 2        # 16 j-tile pairs per i-group

A_SCH = float(8.0 / np.log(2.0))   # fp8e4 Schraudolph scale
B_SCH = 56.0                       # fp8e4 exponent bias * 8

# exp engine split: pair indices (mod 16) sent to DVE-Schraudolph.
# Keep the first pairs of each ig on ACT so deferred boundary work on DVE
# isn't stuck behind an exp.
DVE_EXP = {4, 6, 8, 10, 12, 15}
DVE_EXP_LAST = {4, 6, 8, 10, 12, 15}
DVE_EXP_IG0 = {1, 2, 4, 6, 8, 10, 12, 15}


def build_program():
    nc = bacc.Bacc("TRN2", target_bir_lowering=False, debug=False,
                   enable_asserts=False)

    x_rgb = nc.dram_tensor("x_rgb", [C, NH], F32R, kind="ExternalInput").ap()
    x_ir = nc.dram_tensor("x_ir", [C, N], F32R, kind="ExternalInput").ap()
    wq_bd = nc.dram_tensor("wq_bd", [128, 256], F32R, kind="ExternalInput").ap()
    wk_bd = nc.dram_tensor("wk_bd", [128, 256], F32R, kind="ExternalInput").ap()
    wv_r = nc.dram_tensor("wv_r", [128, 512], F32R, kind="ExternalInput").ap()
    w4t = nc.dram_tensor("w4t", [128, 512], BF16, kind="ExternalInput").ap()
    bq = nc.dram_tensor("bq", [128, 2], F32, kind="ExternalInput").ap()
    bk = nc.dram_tensor("bk", [128, 2], F32, kind="ExternalInput").ap()
    b4 = nc.dram_tensor("b4", [128, 2], F32, kind="ExternalInput").ap()
    out = nc.dram_tensor("out", [C, NH], F32, kind="ExternalOutput").ap()

    with tile.TileContext(nc) as tc:
        with tc.tile_pool(name="persist", bufs=1) as persist:
            qsg = [persist.tile([128, 2, 512], FP8, tag=f"qsg{g}",
                                name=f"qsg{g}") for g in range(IG)]
            ksh = [persist.tile([128, 2, 2048], FP8, tag=f"ksh{h}",
                                name=f"ksh{h}") for h in range(2)]
            # vT2 packs j-tile pairs for DoubleRow: [p, jj, r, c] =
            # v[(16h + 2jj + r)*128 + p, c]; col 256 = ones (softmax denom),
            # cols 257:272 pad for the 16B-aligned r-stride.
            vT2 = [persist.tile([128, 8, 2, 272], FP8, tag=f"vT2{h}",
                                name=f"vT2{h}") for h in range(2)]
            zsg = [[persist.tile([128, 512], BF16, tag=f"zsg{ch}_{g}",
                                 name=f"zsg{ch}_{g}") for g in range(IG)]
                   for ch in range(2)]
            rgbf = [persist.tile([128, NH], F32R, tag=f"rgbf{ch}",
                                 name=f"rgbf{ch}") for ch in range(2)]
            irf = [[persist.tile([128, 2048], F32R, tag=f"irf{ch}_{h}",
                                 name=f"irf{ch}_{h}") for h in range(2)]
                   for ch in range(2)]
            wq_sb = persist.tile([128, 2, 128], F32R, tag="wq_sb", name="wq_sb")
            wk_sb = persist.tile([128, 2, 128], F32R, tag="wk_sb", name="wk_sb")
            wv_sb = persist.tile([128, 2, 256], F32R, tag="wv_sb", name="wv_sb")
            w4_sb = persist.tile([128, 2, 2, 128], BF16, tag="w4_sb", name="w4_sb")
            bq_sb = persist.tile([128, 2], F32, tag="bq_sb", name="bq_sb")
            b4_sb = persist.tile([128, 2], F32, tag="b4_sb", name="b4_sb")
            ident = persist.tile([128, 128], BF16, tag="ident", name="ident")

            # ---- prologue DMAs: critical chunks first (HWDGE serializes
            # issues globally, so minimize count and front-load the head) ----
            nc.sync.dma_start(wk_sb[:], wk_bd)
            nc.sync.dma_start(irf[0][0][:, 0:1024], x_ir[0:128, 0:1024])
            nc.sync.dma_start(irf[1][0][:, 0:1024], x_ir[128:256, 0:1024])
            nc.sync.dma_start(wq_sb[:], wq_bd)
            nc.sync.dma_start(bq_sb[:], bq)
            nc.sync.dma_start(rgbf[0][:, 0:512], x_rgb[0:128, 0:512])
            nc.sync.dma_start(rgbf[1][:, 0:512], x_rgb[128:256, 0:512])
            nc.sync.dma_start(wv_sb[:], wv_r)
            nc.sync.dma_start(irf[0][0][:, 1024:2048], x_ir[0:128, 1024:2048])
            nc.sync.dma_start(irf[1][0][:, 1024:2048], x_ir[128:256, 1024:2048])
            nc.sync.dma_start(irf[0][1][:], x_ir[0:128, 2048:4096])
            nc.sync.dma_start(irf[1][1][:], x_ir[128:256, 2048:4096])
            nc.sync.dma_start(rgbf[0][:, 512:2048], x_rgb[0:128, 512:2048])
            nc.sync.dma_start(rgbf[1][:, 512:2048], x_rgb[128:256, 512:2048])
            nc.sync.dma_start(w4_sb[:], w4t)
            nc.sync.dma_start(b4_sb[:], b4)
            make_identity(nc, ident[:])
            for h in range(2):
                nc.gpsimd.memset(vT2[h][:, :, :, 256:257], 1.0)

            with (
                tc.tile_pool(name="spool", bufs=3, space="PSUM") as spool,
                tc.tile_pool(name="zpool", bufs=2, space="PSUM") as zpool,
                tc.tile_pool(name="pexp", bufs=20) as pexp,
                tc.tile_pool(name="znorm", bufs=8) as znorm,
                tc.tile_pool(name="fin", bufs=2) as fin,
            ):
                # ---- woven gconv slot tasks (share s-pool with pairs) ----
                mv_alt = [0]

                def mover(dst, src, bias=None):
                    mv_alt[0] ^= 1
                    if bias is None:
                        if mv_alt[0]:
                            nc.vector.tensor_copy(dst, src)
                        else:
                            nc.scalar.copy(dst, src)
                    else:
                        if mv_alt[0]:
                            nc.vector.tensor_scalar_add(dst, src, bias)
                        else:
                            nc.scalar.activation(dst, src, AF.Identity,
                                                 bias=bias)

                def kslot(h, ch, half, fast=False):
                    ps = spool.tile([128, 1024], F32, tag="s", name="kps")
                    for q4 in range(2):
                        csl = slice(half * 1024 + q4 * 512,
                                    half * 1024 + (q4 + 1) * 512)
                        nc.tensor.matmul(ps[:, q4 * 512:(q4 + 1) * 512],
                                         wk_sb[:, ch], irf[ch][h][:, csl],
                                         start=True, stop=True)
                        if fast:
                            mover(ksh[h][:, ch, csl],
                                  ps[:, q4 * 512:(q4 + 1) * 512])
                    if not fast:
                        csl = slice(half * 1024, (half + 1) * 1024)
                        mover(ksh[h][:, ch, csl], ps[:])

                def qslot(g):
                    gsl = slice(g * 512, (g + 1) * 512)
                    ps = spool.tile([128, 1024], F32, tag="s", name="qps")
                    for ch in range(2):
                        nc.tensor.matmul(ps[:, ch * 512:(ch + 1) * 512],
                                         wq_sb[:, ch], rgbf[ch][:, gsl],
                                         start=True, stop=True)
                    for ch in range(2):
                        mover(qsg[g][:, ch, :], ps[:, ch * 512:(ch + 1) * 512],
                              bias=bq_sb[:, ch:ch + 1])

                def vslot(h, q):
                    # 4 j-tiles (j = 4q .. 4q+3 within half h) -> vT2
                    ps = spool.tile([128, 1024], F32, tag="s", name="vps")
                    for jl in range(4):
                        j = 4 * q + jl
                        jsl = slice(j * 128, (j + 1) * 128)
                        psl = slice(jl * 256, (jl + 1) * 256)
                        for ch in range(2):
                            nc.tensor.matmul(ps[:, psl], irf[ch][h][:, jsl],
                                             wv_sb[:, ch],
                                             start=(ch == 0), stop=(ch == 1))
                    dst = vT2[h][:, 2 * q:2 * q + 2, :, 0:256]
                    mover(dst, ps[:])

                weave = {
                    0: [lambda: kslot(0, 0, 0), lambda: kslot(0, 1, 0),
                        lambda: qslot(0)],
                    1: [lambda: vslot(0, 0)],
                    2: [lambda: vslot(0, 1)],
                    3: [lambda: kslot(0, 0, 1)],
                    4: [lambda: kslot(0, 1, 1)],
                    5: [lambda: vslot(0, 2), lambda: kslot(1, 0, 0)],
                    6: [lambda: vslot(0, 3), lambda: kslot(1, 1, 0)],
                    7: [lambda: kslot(1, 0, 1)],
                    8: [lambda: kslot(1, 1, 1)],
                    9: [lambda: vslot(1, 0)],
                    10: [lambda: vslot(1, 1)],
                    11: [lambda: vslot(1, 2)],
                    12: [lambda: vslot(1, 3)],
                    13: [lambda: qslot(1)],
                    18: [lambda: qslot(2)],
                    34: [lambda: qslot(3)],
                }

                def phase5_mm(g, oh):
                    ps = zpool.tile([128, 512], F32, tag="z", name="yps")
                    for ch in range(2):
                        nc.tensor.matmul(ps[:], w4_sb[:, ch, oh],
                                         zsg[ch][g][:],
                                         start=(ch == 0), stop=(ch == 1))
                    return ps

                def phase5_chain(g, oh, ps, last):
                    bias = b4_sb[:, oh:oh + 1]
                    # mish(y) = y*tanh(softplus(y)); with u = e^y:
                    # tanh(softplus(y)) = 1 - 2/((u+1)^2+1)
                    chunks = 1
                    cw = 512 // chunks
                    for cix in range(chunks):
                        cs = slice(cix * cw, (cix + 1) * cw)
                        psc = ps[:, cs]
                        u = fin.tile([128, cw], F32, tag=f"u{cix}", name="u")
                        nc.scalar.activation(u[:], psc, AF.Exp, bias=bias)
                        w2 = fin.tile([128, cw], F32, tag=f"w2{cix}", name="w2")
                        nc.scalar.activation(w2[:], u[:], AF.Square, bias=1.0)
                        dd = fin.tile([128, cw], F32, tag=f"dd{cix}", name="dd")
                        t = fin.tile([128, cw], F32, tag=f"t{cix}", name="t")
                        rr = fin.tile([128, cw], F32, tag=f"rr{cix}", name="rr")
                        if last:
                            nc.vector.tensor_scalar_add(dd[:], w2[:], 1.0)
                            nc.vector.reciprocal(rr[:], dd[:])
                            nc.scalar.activation(t[:], rr[:], AF.Identity,
                                                 bias=1.0, scale=-2.0)
                        else:
                            nc.gpsimd.tensor_scalar_add(dd[:], w2[:], 1.0)
                            nc.vector.reciprocal(rr[:], dd[:])
                            nc.gpsimd.tensor_scalar(t[:], rr[:], -2.0, 1.0,
                                                    ALU.mult, ALU.add)
                        m = fin.tile([128, cw], F32, tag=f"m{cix}", name="m")
                        nc.vector.scalar_tensor_tensor(m[:], psc, bias, t[:],
                                                       ALU.add, ALU.mult)
                        o = fin.tile([128, cw], F32, tag=f"o{cix}", name="o")
                        rslice = rgbf[oh][:, g * 512 + cix * cw:
                                          g * 512 + (cix + 1) * cw]
                        if last:
                            nc.vector.tensor_add(o[:], m[:],
                                                 rslice.bitcast(F32))
                        else:
                            nc.gpsimd.tensor_add(o[:], m[:],
                                                 rslice.bitcast(F32))
                        nc.sync.dma_start(
                            out[oh * 128:(oh + 1) * 128,
                                g * 512 + cix * cw:g * 512 + (cix + 1) * cw],
                            o[:])

                deferq = []

                def drain(k):
                    for _ in range(k):
                        if deferq:
                            deferq.pop(0)()

                for ig in range(IG):
                    lastig = ig == IG - 1
                    inline2 = lastig    # last ig: single-pass z, t2/t3 in
                    pend = []           # a long-lived s-pool slot
                    pts = []
                    zps1 = []          # [t0, t1] psum tiles, lazy
                    zps2 = []          # [t2, t3]

                    def flush1(pair, zps1=zps1, zps2=zps2, lastig=inline2):
                        pt, pr = pair
                        if not zps1:
                            zps1.extend(zpool.tile([128, 257], F32, tag="z",
                                                   name=f"zp1_{t}")
                                        for t in range(2))
                        if lastig and not zps2:
                            # last ig: inline single-pass for t2/t3 too, in a
                            # long-lived s-pool slot (bank-aligned regions)
                            z2 = spool.tile([128, 2, 512], F32, tag="s",
                                            name="zp2sp")
                            zps2.extend([(z2[:, tt, 0:257],
                                          z2[:, tt, 256:257],
                                          z2[:, tt, 0:256])
                                         for tt in range(2)])
                        h, jj = pr // 8, pr % 8
                        rhs = vT2[h][:, jj, :, 0:257]
                        for t in range(2):
                            nc.tensor.matmul(
                                zps1[t][:], pt[:, :, t * 128:(t + 1) * 128],
                                rhs, perf_mode=DR,
                                start=(pr == 0), stop=(pr == PAIRS - 1))
                        if lastig:
                            for t in range(2, 4):
                                nc.tensor.matmul(
                                    zps2[t - 2][0],
                                    pt[:, :, t * 128:(t + 1) * 128],
                                    rhs, perf_mode=DR,
                                    start=(pr == 0), stop=(pr == PAIRS - 1))

                    for pr in range(PAIRS):
                        gp = ig * PAIRS + pr
                        for task in weave.get(gp, []):
                            task()
                        ps = spool.tile([128, 2, 512], F32, tag="s", name="sT")
                        for hh in range(2):
                            jt = 2 * pr + hh
                            jsl = slice((jt % 16) * 128, (jt % 16 + 1) * 128)
                            nc.tensor.matmul(ps[:, hh], ksh[jt // 16][:, :, jsl],
                                             qsg[ig][:], perf_mode=DR,
                                             start=True, stop=True)
                        if len(pend) > 2:
                            flush1(pend.pop(0))
                        drain(4)
                        pt = pexp.tile([128, 2, 512], FP8, tag="pt", name="pt")
                        dset = (DVE_EXP_IG0 if ig == 0 else
                                DVE_EXP_LAST if lastig else DVE_EXP)
                        if pr in dset:
                            nc.vector.tensor_scalar(pt[:].bitcast(I8), ps[:],
                                                    A_SCH, B_SCH,
                                                    ALU.mult, ALU.add)
                        else:
                            nc.scalar.activation(pt[:], ps[:], AF.Exp)
                        pend.append((pt, pr))
                        pts.append(pt)

                    # ---- boundary work, deferred into next ig's stream ----
                    def boundary(ig=ig, pend=list(pend), pts=list(pts),
                                 zps1=zps1, zps2=zps2):
                        last = ig == IG - 1
                        rinvs = [None] * 4
                        zns = [None] * 4

                        def ftails():
                            for pair in pend:
                                flush1(pair, zps1=zps1)

                        def zn_one(t, src_den, src_dat):
                            rinvs[t] = znorm.tile([128, 1], F32,
                                                  tag=f"ri{t}", name="ri")
                            nc.vector.reciprocal(rinvs[t][:], src_den)
                            zns[t] = znorm.tile([128, 256], BF16,
                                                tag=f"zn{t}", name="zn")
                            if last and t % 2 == 0:
                                nc.scalar.activation(zns[t][:], src_dat,
                                                     AF.Identity,
                                                     scale=rinvs[t][:, 0:1])
                            else:
                                nc.vector.tensor_scalar_mul(
                                    zns[t][:], src_dat, rinvs[t][:])

                        def norm01():
                            for t in range(2):
                                zn_one(t, zps1[t][:, 256:257],
                                       zps1[t][:, 0:256])

                        def pass2(t):
                            if last:
                                return   # t2/t3 accumulated inline
                            if not zps2:
                                for tt in range(2):
                                    zt = zpool.tile([128, 257], F32, tag="z",
                                                    name=f"zp2_{tt}")
                                    zps2.append((zt[:], zt[:, 256:257],
                                                 zt[:, 0:256]))
                            for pr2, pt2 in enumerate(pts):
                                h, jj = pr2 // 8, pr2 % 8
                                nc.tensor.matmul(
                                    zps2[t - 2][0],
                                    pt2[:, :, t * 128:(t + 1) * 128],
                                    vT2[h][:, jj, :, 0:257], perf_mode=DR,
                                    start=(pr2 == 0), stop=(pr2 == PAIRS - 1))

                        def norm23():
                            for t in range(2, 4):
                                zn_one(t, zps2[t - 2][1], zps2[t - 2][2])

                        def transp(tp2):
                            # i-tiles 2*tp2, 2*tp2+1 for both chunks
                            for ch in range(2):
                                tp = zpool.tile([128, 256], BF16, tag="z",
                                                name="tp")
                                for k in range(2):
                                    t = 2 * tp2 + k
                                    nc.tensor.transpose(
                                        tp[:, k * 128:(k + 1) * 128],
                                        zns[t][:, ch * 128:(ch + 1) * 128],
                                        ident[:])
                                dst = zsg[ch][ig][:,
                                                  tp2 * 256:(tp2 + 1) * 256]
                                if last and ch == 0:
                                    nc.scalar.copy(dst, tp[:])
                                else:
                                    nc.vector.tensor_copy(dst, tp[:])

                        ph5ps = [None, None]

                        def mms():
                            ph5ps[0] = phase5_mm(ig, 0)
                            ph5ps[1] = phase5_mm(ig, 1)

                        if last:
                            return [ftails,
                                    lambda: (norm01(), norm23()),
                                    lambda: transp(0), lambda: transp(1),
                                    mms,
                                    lambda: phase5_chain(ig, 0, ph5ps[0],
                                                         last),
                                    lambda: phase5_chain(ig, 1, ph5ps[1],
                                                         last)]
                        return [ftails, norm01,
                                lambda: transp(0),
                                lambda: pass2(2), lambda: pass2(3),
                                norm23,
                                lambda: transp(1),
                                mms,
                                lambda: phase5_chain(ig, 0, ph5ps[0], last),
                                lambda: phase5_chain(ig, 1, ph5ps[1], last)]

                    deferq.extend(boundary())

                # drain the tail (last ig boundary work)
                while deferq:
                    drain(1)

    nc.finalize()
    return nc


def _blockdiag_T(w, g0, g1):
    """lhsT chunk: [[w[g0].T, 0], [0, w[g1].T]] as [128, 128]."""
    m = np.zeros((128, 128), dtype=np.float64)
    m[:64, :64] = w[g0].T
    m[64:, 64:] = w[g1].T
    return m


def prep_inputs(rgb, ir, w_q, b_q, w_k, b_k, w_v, b_v, w4, b4,
                gamma, beta, rmean, rvar):
    """Host-side prep: fold scale/BN/b_v, pack block-diagonal weights."""
    f64 = np.float64
    w_q, b_q = f64(np.asarray(w_q)), f64(np.asarray(b_q))
    w_k = f64(np.asarray(w_k))
    w_v, b_v = f64(np.asarray(w_v)), f64(np.asarray(b_v))
    w4, b4 = f64(np.asarray(w4)), f64(np.asarray(b4))
    gamma, beta = f64(np.asarray(gamma)), f64(np.asarray(beta))
    rmean, rvar = f64(np.asarray(rmean)), f64(np.asarray(rvar))

    inv = gamma / np.sqrt(rvar + 1e-5)
    w4f = w4 * inv[:, None]                      # BN folded into w4
    b4f = b4 * inv + beta - rmean * inv + w4f @ b_v   # b_v folded

    f32 = np.float32
    bf16 = ml_dtypes.bfloat16
    hs = np.sqrt(SCALE)  # split attention scale between q and k for fp8 range
    wq_bd = np.concatenate([_blockdiag_T(w_q * hs, 0, 1),
                            _blockdiag_T(w_q * hs, 2, 3)], axis=1).astype(f32)
    wk_bd = np.concatenate([_blockdiag_T(w_k * hs, 0, 1),
                            _blockdiag_T(w_k * hs, 2, 3)], axis=1).astype(f32)
    wv_r = np.zeros((128, 512), dtype=np.float64)
    wv_r[:, 0:128] = _blockdiag_T(w_v, 0, 1)
    wv_r[:, 384:512] = _blockdiag_T(w_v, 2, 3)
    wv_r = wv_r.astype(f32)
    w4t = np.zeros((128, 512), dtype=np.float64)
    for ch in range(2):
        for oh in range(2):
            w4t[:, ch * 256 + oh * 128:ch * 256 + (oh + 1) * 128] = \
                w4f[oh * 128:(oh + 1) * 128, ch * 128:(ch + 1) * 128].T
    w4t = w4t.astype(bf16)

    def cols(v):
        return np.stack([v[:128], v[128:]], axis=1).astype(np.float32)

    bq_c = cols(b_q * hs)
    bk_c = np.zeros((128, 2), dtype=np.float32)  # k bias cancels in softmax
    b4_c = cols(b4f)

    rgb_f = np.ascontiguousarray(np.asarray(rgb), dtype=np.float32)
    ir_f = np.ascontiguousarray(np.asarray(ir), dtype=np.float32)

    weights = dict(wq_bd=wq_bd, wk_bd=wk_bd, wv_r=wv_r, w4t=w4t,
                   bq=bq_c, bk=bk_c, b4=b4_c)
    in_maps = []
    for core in range(NCORES):
        b, half = divmod(core, 2)
        x_rgb = np.ascontiguousarray(
            rgb_f[b].reshape(C, N)[:, half * NH:(half + 1) * NH])
        x_ir = np.ascontiguousarray(ir_f[b].reshape(C, N))
        in_maps.append(dict(x_rgb=x_rgb, x_ir=x_ir, **weights))
    return in_maps


_PROGRAM = None


def _get_program():
    global _PROGRAM
    if _PROGRAM is None:
        _PROGRAM = build_program()
    return _PROGRAM


def run(inputs, trace=False, **kw):
    """Run on 8 cores; returns (full_output, BassKernelResults)."""
    nc = _get_program()
    in_maps = prep_inputs(**inputs)
    res = run_bass_kernel_spmd(nc, in_maps, list(range(NCORES)),
                               trace=trace, **kw)
    full = np.zeros((BS, C, H, W), dtype=np.float32)
    for core in range(NCORES):
        b, half = divmod(core, 2)
        full[b].reshape(C, N)[:, half * NH:(half + 1) * NH] = \
            res.results[core]["out"]
    return full, res


def kernel(**inputs) -> np.ndarray:
    out, _ = run(inputs)
    return out


# revision 7
# speedup vs baseline: 1.0330x; 1.0030x over previous
"""Trainium2 Bass kernel v2 for nn_CMF_Block (cross-modal fusion block).

Reference computation (per batch b):
    q = gconv1x1(rgb, w_q, b_q)   # [c, n]   c=256, n=h*w=4096, groups=4
    k = gconv1x1(ir,  w_k, b_k)
    v = gconv1x1(ir,  w_v, b_v)
    attn = softmax(q^T k * c^-0.5, axis=-1)      # [n, n]
    z = v @ attn^T                                # [c, n]
    y = w4 @ z + b4 ; y = BN(y) ; out = rgb + mish(y)

Sharding: 8 cores = 4 batches x 2 query-halves. Each core gets the full
ir slab [256, 4096] plus its rgb query-half [256, 2048] and produces the
matching disjoint output slice [256, 2048]. No collectives.

v2 design vs baseline (129.2us):
  - z matmul (P@V) in fp8e4 DoubleRow (0.5 cyc/col); P written as fp8 by
    the exp stage, vT staged fp8 with 272-stride r-padding (16B dual-fp8
    alignment). Runs as 2 passes of 2 i-tiles so zps needs only 2 psum
    banks, freeing 6 banks for a triple-buffered score pool.
  - softmax exp split across ACT (AF.Exp) and DVE (Schraudolph exp:
    int8(s*8/ln2 + 56) bitcast fp8e4; ~1.5e-4 end-to-end).
  - k bias dropped (constant per query in scores -> cancels in softmax).
  - mish phase mostly on the idle Pool engine (SBUF-only chain).
  - gconv psum shares the score pool slots, woven into the pair stream.
  - i-group boundary work (znorm, pass-2 z, transposes, phase5) is
    deferred and drained into the next i-group's pair stream so no engine
    queue blocks at boundaries; the last group uses a latency-optimized
    ACT/DVE-only phase5 in 256-col chunks.
  - prologue DMAs issue from 4 queues in parallel, critical chunks first.
"""

import sys

sys.path.insert(0, "/opt/trn_rl_repo")

import numpy as np
import ml_dtypes

import concourse.bass as bass
import concourse.tile as tile
from concourse import bacc
from concourse import mybir
from concourse.bass_utils import run_bass_kernel_spmd
from concourse.masks import make_identity

F32 = mybir.dt.float32
F32R = mybir.dt.float32r
BF16 = mybir.dt.bfloat16
FP8 = mybir.dt.float8e4
I8 = mybir.dt.int8
AF = mybir.ActivationFunctionType
DR = mybir.MatmulPerfMode.DoubleRow
ALU = mybir.AluOpType

BS, C, H, W = 4, 256, 64, 64
N = H * W              # 4096
G, CG = 4, 64
NH = N // 2            # 2048 query positions per core
NCORES = 8
SCALE = C ** -0.5      # 1/16

JT = N // 128          # 32 key tiles
IG = 4                 # i-groups of 512 queries
PAIRS = JT // 2        # 16 j-tile pairs per i-group

A_SCH = float(8.0 / np.log(2.0))   # fp8e4 Schraudolph scale
B_SCH = 56.0                       # fp8e4 exponent bias * 8

# exp engine split: pair indices (mod 16) sent to DVE-Schraudolph.
# Keep the first pairs of each ig on ACT so deferred boundary work on DVE
# isn't stuck behind an exp.
DVE_EXP = {4, 6, 8, 10, 12, 15}
DVE_EXP_LAST = {4, 6, 8, 10, 12, 15}
DVE_EXP_IG0 = {1, 2, 3, 4, 6, 8, 10, 12, 15}


def build_program():
    nc = bacc.Bacc("TRN2", target_bir_lowering=False, debug=False,
                   enable_asserts=False)

    x_rgb = nc.dram_tensor("x_rgb", [C, NH], F32R, kind="ExternalInput").ap()
    x_ir = nc.dram_tensor("x_ir", [C, N], F32R, kind="ExternalInput").ap()
    wq_bd = nc.dram_tensor("wq_bd", [128, 256], F32R, kind="ExternalInput").ap()
    wk_bd = nc.dram_tensor("wk_bd", [128, 256], F32R, kind="ExternalInput").ap()
    wv_r = nc.dram_tensor("wv_r", [128, 512], F32R, kind="ExternalInput").ap()
    w4t = nc.dram_tensor("w4t", [128, 512], BF16, kind="ExternalInput").ap()
    bq = nc.dram_tensor("bq", [128, 2], F32, kind="ExternalInput").ap()
    bk = nc.dram_tensor("bk", [128, 2], F32, kind="ExternalInput").ap()
    b4 = nc.dram_tensor("b4", [128, 2], F32, kind="ExternalInput").ap()
    out = nc.dram_tensor("out", [C, NH], F32, kind="ExternalOutput").ap()

    with tile.TileContext(nc) as tc:
        with tc.tile_pool(name="persist", bufs=1) as persist:
            qsg = [persist.tile([128, 2, 512], FP8, tag=f"qsg{g}",
                                name=f"qsg{g}") for g in range(IG)]
            ksh = [persist.tile([128, 2, 2048], FP8, tag=f"ksh{h}",
                                name=f"ksh{h}") for h in range(2)]
            # vT2 packs j-tile pairs for DoubleRow: [p, jj, r, c] =
            # v[(16h + 2jj + r)*128 + p, c]; col 256 = ones (softmax denom),
            # cols 257:272 pad for the 16B-aligned r-stride.
            vT2 = [persist.tile([128, 8, 2, 272], FP8, tag=f"vT2{h}",
                                name=f"vT2{h}") for h in range(2)]
            zsg = [[persist.tile([128, 512], BF16, tag=f"zsg{ch}_{g}",
                                 name=f"zsg{ch}_{g}") for g in range(IG)]
                   for ch in range(2)]
            rgbf = [persist.tile([128, NH], F32R, tag=f"rgbf{ch}",
                                 name=f"rgbf{ch}") for ch in range(2)]
            irf = [[persist.tile([128, 2048], F32R, tag=f"irf{ch}_{h}",
                                 name=f"irf{ch}_{h}") for h in range(2)]
                   for ch in range(2)]
            wq_sb = persist.tile([128, 2, 128], F32R, tag="wq_sb", name="wq_sb")
            wk_sb = persist.tile([128, 2, 128], F32R, tag="wk_sb", name="wk_sb")
            wv_sb = persist.tile([128, 2, 256], F32R, tag="wv_sb", name="wv_sb")
            w4_sb = persist.tile([128, 2, 2, 128], BF16, tag="w4_sb", name="w4_sb")
            bq_sb = persist.tile([128, 2], F32, tag="bq_sb", name="bq_sb")
            b4_sb = persist.tile([128, 2], F32, tag="b4_sb", name="b4_sb")
            ident = persist.tile([128, 128], BF16, tag="ident", name="ident")

            # ---- prologue DMAs: critical chunks first (HWDGE serializes
            # issues globally, so minimize count and front-load the head) ----
            nc.sync.dma_start(wk_sb[:], wk_bd)
            nc.sync.dma_start(irf[0][0][:, 0:1024], x_ir[0:128, 0:1024])
            nc.sync.dma_start(irf[1][0][:, 0:1024], x_ir[128:256, 0:1024])
            nc.sync.dma_start(wq_sb[:], wq_bd)
            nc.sync.dma_start(bq_sb[:], bq)
            nc.sync.dma_start(rgbf[0][:, 0:512], x_rgb[0:128, 0:512])
            nc.sync.dma_start(rgbf[1][:, 0:512], x_rgb[128:256, 0:512])
            nc.sync.dma_start(wv_sb[:], wv_r)
            nc.sync.dma_start(irf[0][0][:, 1024:2048], x_ir[0:128, 1024:2048])
            nc.sync.dma_start(irf[1][0][:, 1024:2048], x_ir[128:256, 1024:2048])
            nc.sync.dma_start(irf[0][1][:], x_ir[0:128, 2048:4096])
            nc.sync.dma_start(irf[1][1][:], x_ir[128:256, 2048:4096])
            nc.sync.dma_start(rgbf[0][:, 512:2048], x_rgb[0:128, 512:2048])
            nc.sync.dma_start(rgbf[1][:, 512:2048], x_rgb[128:256, 512:2048])
            nc.sync.dma_start(w4_sb[:], w4t)
            nc.sync.dma_start(b4_sb[:], b4)
            make_identity(nc, ident[:])
            for h in range(2):
                nc.gpsimd.memset(vT2[h][:, :, :, 256:257], 1.0)

            with (
                tc.tile_pool(name="spool", bufs=3, space="PSUM") as spool,
                tc.tile_pool(name="zpool", bufs=2, space="PSUM") as zpool,
                tc.tile_pool(name="pexp", bufs=20) as pexp,
                tc.tile_pool(name="znorm", bufs=8) as znorm,
                tc.tile_pool(name="fin", bufs=2) as fin,
            ):
                # ---- woven gconv slot tasks (share s-pool with pairs) ----
                mv_alt = [0]

                def mover(dst, src, bias=None):
                    mv_alt[0] ^= 1
                    if bias is None:
                        if mv_alt[0]:
                            nc.vector.tensor_copy(dst, src)
                        else:
                            nc.scalar.copy(dst, src)
                    else:
                        if mv_alt[0]:
                            nc.vector.tensor_scalar_add(dst, src, bias)
                        else:
                            nc.scalar.activation(dst, src, AF.Identity,
                                                 bias=bias)

                def kslot(h, ch, half, fast=False):
                    ps = spool.tile([128, 1024], F32, tag="s", name="kps")
                    for q4 in range(2):
                        csl = slice(half * 1024 + q4 * 512,
                                    half * 1024 + (q4 + 1) * 512)
                        nc.tensor.matmul(ps[:, q4 * 512:(q4 + 1) * 512],
                                         wk_sb[:, ch], irf[ch][h][:, csl],
                                         start=True, stop=True)
                        if fast:
                            mover(ksh[h][:, ch, csl],
                                  ps[:, q4 * 512:(q4 + 1) * 512])
                    if not fast:
                        csl = slice(half * 1024, (half + 1) * 1024)
                        mover(ksh[h][:, ch, csl], ps[:])

                def qslot(g):
                    gsl = slice(g * 512, (g + 1) * 512)
                    ps = spool.tile([128, 1024], F32, tag="s", name="qps")
                    for ch in range(2):
                        nc.tensor.matmul(ps[:, ch * 512:(ch + 1) * 512],
                                         wq_sb[:, ch], rgbf[ch][:, gsl],
                                         start=True, stop=True)
                    for ch in range(2):
                        mover(qsg[g][:, ch, :], ps[:, ch * 512:(ch + 1) * 512],
                              bias=bq_sb[:, ch:ch + 1])

                def vslot(h, q):
                    # 4 j-tiles (j = 4q .. 4q+3 within half h) -> vT2
                    ps = spool.tile([128, 1024], F32, tag="s", name="vps")
                    for jl in range(4):
                        j = 4 * q + jl
                        jsl = slice(j * 128, (j + 1) * 128)
                        psl = slice(jl * 256, (jl + 1) * 256)
                        for ch in range(2):
                            nc.tensor.matmul(ps[:, psl], irf[ch][h][:, jsl],
                                             wv_sb[:, ch],
                                             start=(ch == 0), stop=(ch == 1))
                    dst = vT2[h][:, 2 * q:2 * q + 2, :, 0:256]
                    mover(dst, ps[:])

                weave = {
                    0: [lambda: kslot(0, 0, 0), lambda: kslot(0, 1, 0),
                        lambda: qslot(0)],
                    1: [lambda: vslot(0, 0)],
                    2: [lambda: vslot(0, 1)],
                    3: [lambda: kslot(0, 0, 1)],
                    4: [lambda: kslot(0, 1, 1)],
                    5: [lambda: vslot(0, 2), lambda: kslot(1, 0, 0)],
                    6: [lambda: vslot(0, 3), lambda: kslot(1, 1, 0)],
                    7: [lambda: kslot(1, 0, 1)],
                    8: [lambda: kslot(1, 1, 1)],
                    9: [lambda: vslot(1, 0)],
                    10: [lambda: vslot(1, 1)],
                    11: [lambda: vslot(1, 2)],
                    12: [lambda: vslot(1, 3)],
                    13: [lambda: qslot(1)],
                    18: [lambda: qslot(2)],
                    34: [lambda: qslot(3)],
                }

                def phase5_mm(g, oh):
                    ps = zpool.tile([128, 512], F32, tag="z", name="yps")
                    for ch in range(2):
                        nc.tensor.matmul(ps[:], w4_sb[:, ch, oh],
                                         zsg[ch][g][:],
                                         start=(ch == 0), stop=(ch == 1))
                    return ps

                def phase5_chain(g, oh, ps, last):
                    bias = b4_sb[:, oh:oh + 1]
                    # mish(y) = y*tanh(softplus(y)); with u = e^y:
                    # tanh(softplus(y)) = 1 - 2/((u+1)^2+1)
                    chunks = 1
                    cw = 512 // chunks
                    for cix in range(chunks):
                        cs = slice(cix * cw, (cix + 1) * cw)
                        psc = ps[:, cs]
                        u = fin.tile([128, cw], F32, tag=f"u{cix}", name="u")
                        nc.scalar.activation(u[:], psc, AF.Exp, bias=bias)
                        w2 = fin.tile([128, cw], F32, tag=f"w2{cix}", name="w2")
                        nc.scalar.activation(w2[:], u[:], AF.Square, bias=1.0)
                        dd = fin.tile([128, cw], F32, tag=f"dd{cix}", name="dd")
                        t = fin.tile([128, cw], F32, tag=f"t{cix}", name="t")
                        rr = fin.tile([128, cw], F32, tag=f"rr{cix}", name="rr")
                        if last:
                            nc.vector.tensor_scalar_add(dd[:], w2[:], 1.0)
                            nc.vector.reciprocal(rr[:], dd[:])
                            nc.scalar.activation(t[:], rr[:], AF.Identity,
                                                 bias=1.0, scale=-2.0)
                        else:
                            nc.gpsimd.tensor_scalar_add(dd[:], w2[:], 1.0)
                            nc.vector.reciprocal(rr[:], dd[:])
                            nc.gpsimd.tensor_scalar(t[:], rr[:], -2.0, 1.0,
                                                    ALU.mult, ALU.add)
                        m = fin.tile([128, cw], F32, tag=f"m{cix}", name="m")
                        nc.vector.scalar_tensor_tensor(m[:], psc, bias, t[:],
                                                       ALU.add, ALU.mult)
                        o = fin.tile([128, cw], F32, tag=f"o{cix}", name="o")
                        rslice = rgbf[oh][:, g * 512 + cix * cw:
                                          g * 512 + (cix + 1) * cw]
                        if last:
                            nc.vector.tensor_add(o[:], m[:],
                                                 rslice.bitcast(F32))
                        else:
                            nc.gpsimd.tensor_add(o[:], m[:],
                                                 rslice.bitcast(F32))
                        nc.sync.dma_start(
                            out[oh * 128:(oh + 1) * 128,
                                g * 512 + cix * cw:g * 512 + (cix + 1) * cw],
                            o[:])

                deferq = []

                def drain(k):
                    for _ in range(k):
                        if deferq:
                            deferq.pop(0)()

                for ig in range(IG):
                    lastig = ig == IG - 1
                    inline2 = lastig    # last ig: single-pass z, t2/t3 in
                    pend = []           # a long-lived s-pool slot
                    pts = []
                    zps1 = []          # [t0, t1] psum tiles, lazy
                    zps2 = []          # [t2, t3]

                    def flush1(pair, zps1=zps1, zps2=zps2, lastig=inline2):
                        pt, pr = pair
                        if not zps1:
                            zps1.extend(zpool.tile([128, 257], F32, tag="z",
                                                   name=f"zp1_{t}")
                                        for t in range(2))
                        if lastig and not zps2:
                            # last ig: inline single-pass for t2/t3 too, in a
                            # long-lived s-pool slot (bank-aligned regions)
                            z2 = spool.tile([128, 2, 512], F32, tag="s",
                                            name="zp2sp")
                            zps2.extend([(z2[:, tt, 0:257],
                                          z2[:, tt, 256:257],
                                          z2[:, tt, 0:256])
                                         for tt in range(2)])
                        h, jj = pr // 8, pr % 8
                        rhs = vT2[h][:, jj, :, 0:257]
                        for t in range(2):
                            nc.tensor.matmul(
                                zps1[t][:], pt[:, :, t * 128:(t + 1) * 128],
                                rhs, perf_mode=DR,
                                start=(pr == 0), stop=(pr == PAIRS - 1))
                        if lastig:
                            for t in range(2, 4):
                                nc.tensor.matmul(
                                    zps2[t - 2][0],
                                    pt[:, :, t * 128:(t + 1) * 128],
                                    rhs, perf_mode=DR,
                                    start=(pr == 0), stop=(pr == PAIRS - 1))

                    for pr in range(PAIRS):
                        gp = ig * PAIRS + pr
                        for task in weave.get(gp, []):
                            task()
                        ps = spool.tile([128, 2, 512], F32, tag="s", name="sT")
                        for hh in range(2):
                            jt = 2 * pr + hh
                            jsl = slice((jt % 16) * 128, (jt % 16 + 1) * 128)
                            nc.tensor.matmul(ps[:, hh], ksh[jt // 16][:, :, jsl],
                                             qsg[ig][:], perf_mode=DR,
                                             start=True, stop=True)
                        if len(pend) > 2:
                            flush1(pend.pop(0))
                        drain(4)
                        pt = pexp.tile([128, 2, 512], FP8, tag="pt", name="pt")
                        dset = (DVE_EXP_IG0 if ig == 0 else
                                DVE_EXP_LAST if lastig else DVE_EXP)
                        if pr in dset:
                            nc.vector.tensor_scalar(pt[:].bitcast(I8), ps[:],
                                                    A_SCH, B_SCH,
                                                    ALU.mult, ALU.add)
                        else:
                            nc.scalar.activation(pt[:], ps[:], AF.Exp)
                        pend.append((pt, pr))
                        pts.append(pt)

                    # ---- boundary work, deferred into next ig's stream ----
                    def boundary(ig=ig, pend=list(pend), pts=list(pts),
                                 zps1=zps1, zps2=zps2):
                        last = ig == IG - 1
                        rinvs = [None] * 4
                        zns = [None] * 4

                        def ftails():
                            for pair in pend:
                                flush1(pair, zps1=zps1)

                        def zn_one(t, src_den, src_dat):
                            rinvs[t] = znorm.tile([128, 1], F32,
                                                  tag=f"ri{t}", name="ri")
                            nc.vector.reciprocal(rinvs[t][:], src_den)
                            zns[t] = znorm.tile([128, 256], BF16,
                                                tag=f"zn{t}", name="zn")
                            if last and t % 2 == 0:
                                nc.scalar.activation(zns[t][:], src_dat,
                                                     AF.Identity,
                                                     scale=rinvs[t][:, 0:1])
                            else:
                                nc.vector.tensor_scalar_mul(
                                    zns[t][:], src_dat, rinvs[t][:])

                        def norm01():
                            for t in range(2):
                                zn_one(t, zps1[t][:, 256:257],
                                       zps1[t][:, 0:256])

                        def pass2(t):
                            if last:
                                return   # t2/t3 accumulated inline
                            if not zps2:
                                for tt in range(2):
                                    zt = zpool.tile([128, 257], F32, tag="z",
                                                    name=f"zp2_{tt}")
                                    zps2.append((zt[:], zt[:, 256:257],
                                                 zt[:, 0:256]))
                            for pr2, pt2 in enumerate(pts):
                                h, jj = pr2 // 8, pr2 % 8
                                nc.tensor.matmul(
                                    zps2[t - 2][0],
                                    pt2[:, :, t * 128:(t + 1) * 128],
                                    vT2[h][:, jj, :, 0:257], perf_mode=DR,
                                    start=(pr2 == 0), stop=(pr2 == PAIRS - 1))

                        def norm23():
                            for t in range(2, 4):
                                zn_one(t, zps2[t - 2][1], zps2[t - 2][2])

                        def transp(tp2):
                            # i-tiles 2*tp2, 2*tp2+1 for both chunks
                            for ch in range(2):
                                tp = zpool.tile([128, 256], BF16, tag="z",
                                                name="tp")
                                for k in range(2):
                                    t = 2 * tp2 + k
                                    nc.tensor.transpose(
                                        tp[:, k * 128:(k + 1) * 128],
                                        zns[t][:, ch * 128:(ch + 1) * 128],
                                        ident[:])
                                dst = zsg[ch][ig][:,
                                                  tp2 * 256:(tp2 + 1) * 256]
                                if last and ch == 0:
                                    nc.scalar.copy(dst, tp[:])
                                else:
                                    nc.vector.tensor_copy(dst, tp[:])

                        ph5ps = [None, None]

                        def mms():
                            ph5ps[0] = phase5_mm(ig, 0)
                            ph5ps[1] = phase5_mm(ig, 1)

                        if last:
                            return [ftails,
                                    lambda: (norm01(), norm23()),
                                    lambda: transp(0), lambda: transp(1),
                                    mms,
                                    lambda: phase5_chain(ig, 0, ph5ps[0],
                                                         last),
                                    lambda: phase5_chain(ig, 1, ph5ps[1],
                                                         last)]
                        return [ftails, norm01,
                                lambda: transp(0),
                                lambda: pass2(2), lambda: pass2(3),
                                norm23,
                                lambda: transp(1),
                                mms,
                                lambda: phase5_chain(ig, 0, ph5ps[0], last),
                                lambda: phase5_chain(ig, 1, ph5ps[1], last)]

                    deferq.extend(boundary())

                # drain the tail (last ig boundary work)
                while deferq:
                    drain(1)

    nc.finalize()
    return nc


def _blockdiag_T(w, g0, g1):
    """lhsT chunk: [[w[g0].T, 0], [0, w[g1].T]] as [128, 128]."""
    m = np.zeros((128, 128), dtype=np.float64)
    m[:64, :64] = w[g0].T
    m[64:, 64:] = w[g1].T
    return m


def prep_inputs(rgb, ir, w_q, b_q, w_k, b_k, w_v, b_v, w4, b4,
                gamma, beta, rmean, rvar):
    """Host-side prep: fold scale/BN/b_v, pack block-diagonal weights."""
    f64 = np.float64
    w_q, b_q = f64(np.asarray(w_q)), f64(np.asarray(b_q))
    w_k = f64(np.asarray(w_k))
    w_v, b_v = f64(np.asarray(w_v)), f64(np.asarray(b_v))
    w4, b4 = f64(np.asarray(w4)), f64(np.asarray(b4))
    gamma, beta = f64(np.asarray(gamma)), f64(np.asarray(beta))
    rmean, rvar = f64(np.asarray(rmean)), f64(np.asarray(rvar))

    inv = gamma / np.sqrt(rvar + 1e-5)
    w4f = w4 * inv[:, None]                      # BN folded into w4
    b4f = b4 * inv + beta - rmean * inv + w4f @ b_v   # b_v folded

    f32 = np.float32
    bf16 = ml_dtypes.bfloat16
    hs = np.sqrt(SCALE)  # split attention scale between q and k for fp8 range
    wq_bd = np.concatenate([_blockdiag_T(w_q * hs, 0, 1),
                            _blockdiag_T(w_q * hs, 2, 3)], axis=1).astype(f32)
    wk_bd = np.concatenate([_blockdiag_T(w_k * hs, 0, 1),
                            _blockdiag_T(w_k * hs, 2, 3)], axis=1).astype(f32)
    wv_r = np.zeros((128, 512), dtype=np.float64)
    wv_r[:, 0:128] = _blockdiag_T(w_v, 0, 1)
    wv_r[:, 384:512] = _blockdiag_T(w_v, 2, 3)
    wv_r = wv_r.astype(f32)
    w4t = np.zeros((128, 512), dtype=np.float64)
    for ch in range(2):
        for oh in range(2):
            w4t[:, ch * 256 + oh * 128:ch * 256 + (oh + 1) * 128] = \
                w4f[oh * 128:(oh + 1) * 128, ch * 128:(ch + 1) * 128].T
    w4t = w4t.astype(bf16)

    def cols(v):
        return np.stack([v[:128], v[128:]], axis=1).astype(np.float32)

    bq_c = cols(b_q * hs)
    bk_c = np.zeros((128, 2), dtype=np.float32)  # k bias cancels in softmax
    b4_c = cols(b4f)

    rgb_f = np.ascontiguousarray(np.asarray(rgb), dtype=np.float32)
    ir_f = np.ascontiguousarray(np.asarray(ir), dtype=np.float32)

    weights = dict(wq_bd=wq_bd, wk_bd=wk_bd, wv_r=wv_r, w4t=w4t,
                   bq=bq_c, bk=bk_c, b4=b4_c)
    in_maps = []
    for core in range(NCORES):
        b, half = divmod(core, 2)
        x_rgb = np.ascontiguousarray(
            rgb_f[b].reshape(C, N)[:, half * NH:(half + 1) * NH])
        x_ir = np.ascontiguousarray(ir_f[b].reshape(C, N))
        in_maps.append(dict(x_rgb=x_rgb, x_ir=x_ir, **weights))
    return in_maps


_PROGRAM = None


def _get_program():
    global _PROGRAM
    if _PROGRAM is None:
        _PROGRAM = build_program()
    return _PROGRAM


def run(inputs, trace=False, **kw):
    """Run on 8 cores; returns (full_output, BassKernelResults)."""
    nc = _get_program()
    in_maps = prep_inputs(**inputs)
    res = run_bass_kernel_spmd(nc, in_maps, list(range(NCORES)),
                               trace=trace, **kw)
    full = np.zeros((BS, C, H, W), dtype=np.float32)
    for core in range(NCORES):
        b, half = divmod(core, 2)
        full[b].reshape(C, N)[:, half * NH:(half + 1) * NH] = \
            res.results[core]["out"]
    return full, res


def kernel(**inputs) -> np.ndarray:
    out, _ = run(inputs)
    return out


# revision 8
# speedup vs baseline: 1.0357x; 1.0026x over previous
"""Trainium2 Bass kernel v2 for nn_CMF_Block (cross-modal fusion block).

Reference computation (per batch b):
    q = gconv1x1(rgb, w_q, b_q)   # [c, n]   c=256, n=h*w=4096, groups=4
    k = gconv1x1(ir,  w_k, b_k)
    v = gconv1x1(ir,  w_v, b_v)
    attn = softmax(q^T k * c^-0.5, axis=-1)      # [n, n]
    z = v @ attn^T                                # [c, n]
    y = w4 @ z + b4 ; y = BN(y) ; out = rgb + mish(y)

Sharding: 8 cores = 4 batches x 2 query-halves. Each core gets the full
ir slab [256, 4096] plus its rgb query-half [256, 2048] and produces the
matching disjoint output slice [256, 2048]. No collectives.

v2 design vs baseline (129.2us):
  - z matmul (P@V) in fp8e4 DoubleRow (0.5 cyc/col); P written as fp8 by
    the exp stage, vT staged fp8 with 272-stride r-padding (16B dual-fp8
    alignment). Runs as 2 passes of 2 i-tiles so zps needs only 2 psum
    banks, freeing 6 banks for a triple-buffered score pool.
  - softmax exp split across ACT (AF.Exp) and DVE (Schraudolph exp:
    int8(s*8/ln2 + 56) bitcast fp8e4; ~1.5e-4 end-to-end).
  - k bias dropped (constant per query in scores -> cancels in softmax).
  - mish phase mostly on the idle Pool engine (SBUF-only chain).
  - gconv psum shares the score pool slots, woven into the pair stream.
  - i-group boundary work (znorm, pass-2 z, transposes, phase5) is
    deferred and drained into the next i-group's pair stream so no engine
    queue blocks at boundaries; the last group uses a latency-optimized
    ACT/DVE-only phase5 in 256-col chunks.
  - prologue DMAs issue from 4 queues in parallel, critical chunks first.
"""

import sys

sys.path.insert(0, "/opt/trn_rl_repo")

import numpy as np
import ml_dtypes

import concourse.bass as bass
import concourse.tile as tile
from concourse import bacc
from concourse import mybir
from concourse.bass_utils import run_bass_kernel_spmd
from concourse.masks import make_identity

F32 = mybir.dt.float32
F32R = mybir.dt.float32r
BF16 = mybir.dt.bfloat16
FP8 = mybir.dt.float8e4
I8 = mybir.dt.int8
AF = mybir.ActivationFunctionType
DR = mybir.MatmulPerfMode.DoubleRow
ALU = mybir.AluOpType

BS, C, H, W = 4, 256, 64, 64
N = H * W              # 4096
G, CG = 4, 64
NH = N // 2            # 2048 query positions per core
NCORES = 8
SCALE = C ** -0.5      # 1/16

JT = N // 128          # 32 key tiles
IG = 4                 # i-groups of 512 queries
PAIRS = JT // 2        # 16 j-tile pairs per i-group

A_SCH = float(8.0 / np.log(2.0))   # fp8e4 Schraudolph scale
B_SCH = 56.0                       # fp8e4 exponent bias * 8

# exp engine split: pair indices (mod 16) sent to DVE-Schraudolph.
# Keep the first pairs of each ig on ACT so deferred boundary work on DVE
# isn't stuck behind an exp.
DVE_EXP = {4, 6, 8, 10, 12, 15}
DVE_EXP_LAST = {4, 6, 8, 10, 12, 15}
DVE_EXP_IG0 = {1, 2, 3, 4, 6, 8, 10, 12, 15}


def build_program():
    nc = bacc.Bacc("TRN2", target_bir_lowering=False, debug=False,
                   enable_asserts=False)

    x_rgb = nc.dram_tensor("x_rgb", [C, NH], F32R, kind="ExternalInput").ap()
    x_ir = nc.dram_tensor("x_ir", [C, N], F32R, kind="ExternalInput").ap()
    wq_bd = nc.dram_tensor("wq_bd", [128, 256], F32R, kind="ExternalInput").ap()
    wk_bd = nc.dram_tensor("wk_bd", [128, 256], F32R, kind="ExternalInput").ap()
    wv_r = nc.dram_tensor("wv_r", [128, 512], F32R, kind="ExternalInput").ap()
    w4t = nc.dram_tensor("w4t", [128, 512], BF16, kind="ExternalInput").ap()
    bq = nc.dram_tensor("bq", [128, 2], F32, kind="ExternalInput").ap()
    bk = nc.dram_tensor("bk", [128, 2], F32, kind="ExternalInput").ap()
    b4 = nc.dram_tensor("b4", [128, 2], F32, kind="ExternalInput").ap()
    out = nc.dram_tensor("out", [C, NH], F32, kind="ExternalOutput").ap()

    with tile.TileContext(nc) as tc:
        with tc.tile_pool(name="persist", bufs=1) as persist:
            qsg = [persist.tile([128, 2, 512], FP8, tag=f"qsg{g}",
                                name=f"qsg{g}") for g in range(IG)]
            ksh = [persist.tile([128, 2, 2048], FP8, tag=f"ksh{h}",
                                name=f"ksh{h}") for h in range(2)]
            # vT2 packs j-tile pairs for DoubleRow: [p, jj, r, c] =
            # v[(16h + 2jj + r)*128 + p, c]; col 256 = ones (softmax denom),
            # cols 257:272 pad for the 16B-aligned r-stride.
            vT2 = [persist.tile([128, 8, 2, 272], FP8, tag=f"vT2{h}",
                                name=f"vT2{h}") for h in range(2)]
            zsg = [[persist.tile([128, 512], BF16, tag=f"zsg{ch}_{g}",
                                 name=f"zsg{ch}_{g}") for g in range(IG)]
                   for ch in range(2)]
            rgbf = [persist.tile([128, NH], F32R, tag=f"rgbf{ch}",
                                 name=f"rgbf{ch}") for ch in range(2)]
            irf = [[persist.tile([128, 2048], F32R, tag=f"irf{ch}_{h}",
                                 name=f"irf{ch}_{h}") for h in range(2)]
                   for ch in range(2)]
            wq_sb = persist.tile([128, 2, 128], F32R, tag="wq_sb", name="wq_sb")
            wk_sb = persist.tile([128, 2, 128], F32R, tag="wk_sb", name="wk_sb")
            wv_sb = persist.tile([128, 2, 256], F32R, tag="wv_sb", name="wv_sb")
            w4_sb = persist.tile([128, 2, 2, 128], BF16, tag="w4_sb", name="w4_sb")
            bq_sb = persist.tile([128, 2], F32, tag="bq_sb", name="bq_sb")
            b4_sb = persist.tile([128, 2], F32, tag="b4_sb", name="b4_sb")
            ident = persist.tile([128, 128], BF16, tag="ident", name="ident")

            # ---- prologue DMAs: critical chunks first (HWDGE serializes
            # issues globally, so minimize count and front-load the head) ----
            nc.sync.dma_start(wk_sb[:], wk_bd)
            nc.sync.dma_start(irf[0][0][:, 0:1024], x_ir[0:128, 0:1024])
            nc.sync.dma_start(irf[1][0][:, 0:1024], x_ir[128:256, 0:1024])
            nc.sync.dma_start(wq_sb[:], wq_bd)
            nc.sync.dma_start(bq_sb[:], bq)
            nc.sync.dma_start(rgbf[0][:, 0:512], x_rgb[0:128, 0:512])
            nc.sync.dma_start(rgbf[1][:, 0:512], x_rgb[128:256, 0:512])
            nc.sync.dma_start(wv_sb[:], wv_r)
            nc.sync.dma_start(irf[0][0][:, 1024:2048], x_ir[0:128, 1024:2048])
            nc.sync.dma_start(irf[1][0][:, 1024:2048], x_ir[128:256, 1024:2048])
            nc.sync.dma_start(irf[0][1][:], x_ir[0:128, 2048:4096])
            nc.sync.dma_start(irf[1][1][:], x_ir[128:256, 2048:4096])
            nc.sync.dma_start(rgbf[0][:, 512:2048], x_rgb[0:128, 512:2048])
            nc.sync.dma_start(rgbf[1][:, 512:2048], x_rgb[128:256, 512:2048])
            nc.sync.dma_start(w4_sb[:], w4t)
            nc.sync.dma_start(b4_sb[:], b4)
            make_identity(nc, ident[:])
            for h in range(2):
                nc.gpsimd.memset(vT2[h][:, :, :, 256:257], 1.0)

            with (
                tc.tile_pool(name="spool", bufs=3, space="PSUM") as spool,
                tc.tile_pool(name="zpool", bufs=2, space="PSUM") as zpool,
                tc.tile_pool(name="pexp", bufs=20) as pexp,
                tc.tile_pool(name="znorm", bufs=8) as znorm,
                tc.tile_pool(name="fin", bufs=2) as fin,
            ):
                # ---- woven gconv slot tasks (share s-pool with pairs) ----
                mv_alt = [0]

                def mover(dst, src, bias=None):
                    mv_alt[0] ^= 1
                    if bias is None:
                        if mv_alt[0]:
                            nc.vector.tensor_copy(dst, src)
                        else:
                            nc.scalar.copy(dst, src)
                    else:
                        if mv_alt[0]:
                            nc.vector.tensor_scalar_add(dst, src, bias)
                        else:
                            nc.scalar.activation(dst, src, AF.Identity,
                                                 bias=bias)

                def kslot(h, ch, half, fast=False):
                    ps = spool.tile([128, 1024], F32, tag="s", name="kps")
                    for q4 in range(2):
                        csl = slice(half * 1024 + q4 * 512,
                                    half * 1024 + (q4 + 1) * 512)
                        nc.tensor.matmul(ps[:, q4 * 512:(q4 + 1) * 512],
                                         wk_sb[:, ch], irf[ch][h][:, csl],
                                         start=True, stop=True)
                        if fast:
                            mover(ksh[h][:, ch, csl],
                                  ps[:, q4 * 512:(q4 + 1) * 512])
                    if not fast:
                        csl = slice(half * 1024, (half + 1) * 1024)
                        mover(ksh[h][:, ch, csl], ps[:])

                def qslot(g):
                    gsl = slice(g * 512, (g + 1) * 512)
                    ps = spool.tile([128, 1024], F32, tag="s", name="qps")
                    for ch in range(2):
                        nc.tensor.matmul(ps[:, ch * 512:(ch + 1) * 512],
                                         wq_sb[:, ch], rgbf[ch][:, gsl],
                                         start=True, stop=True)
                    nc.vector.tensor_scalar_add(
                        qsg[g][:, 0, :], ps[:, 0:512], bq_sb[:, 0:1])
                    nc.vector.tensor_scalar_add(
                        qsg[g][:, 1, :], ps[:, 512:1024], bq_sb[:, 1:2])

                def vslot(h, q):
                    # 4 j-tiles (j = 4q .. 4q+3 within half h) -> vT2
                    ps = spool.tile([128, 1024], F32, tag="s", name="vps")
                    for jl in range(4):
                        j = 4 * q + jl
                        jsl = slice(j * 128, (j + 1) * 128)
                        psl = slice(jl * 256, (jl + 1) * 256)
                        for ch in range(2):
                            nc.tensor.matmul(ps[:, psl], irf[ch][h][:, jsl],
                                             wv_sb[:, ch],
                                             start=(ch == 0), stop=(ch == 1))
                    dst = vT2[h][:, 2 * q:2 * q + 2, :, 0:256]
                    mover(dst, ps[:])

                weave = {
                    0: [lambda: kslot(0, 0, 0), lambda: kslot(0, 1, 0),
                        lambda: qslot(0)],
                    1: [lambda: vslot(0, 0)],
                    2: [lambda: vslot(0, 1)],
                    3: [lambda: kslot(0, 0, 1)],
                    4: [lambda: kslot(0, 1, 1)],
                    5: [lambda: vslot(0, 2), lambda: kslot(1, 0, 0)],
                    6: [lambda: vslot(0, 3), lambda: kslot(1, 1, 0)],
                    7: [lambda: kslot(1, 0, 1)],
                    8: [lambda: kslot(1, 1, 1)],
                    9: [lambda: vslot(1, 0)],
                    10: [lambda: vslot(1, 1)],
                    11: [lambda: vslot(1, 2)],
                    12: [lambda: vslot(1, 3)],
                    13: [lambda: qslot(1)],
                    18: [lambda: qslot(2)],
                    34: [lambda: qslot(3)],
                }

                def phase5_mm(g, oh):
                    ps = zpool.tile([128, 512], F32, tag="z", name="yps")
                    for ch in range(2):
                        nc.tensor.matmul(ps[:], w4_sb[:, ch, oh],
                                         zsg[ch][g][:],
                                         start=(ch == 0), stop=(ch == 1))
                    return ps

                def phase5_chain(g, oh, ps, last):
                    bias = b4_sb[:, oh:oh + 1]
                    # mish(y) = y*tanh(softplus(y)); with u = e^y:
                    # tanh(softplus(y)) = 1 - 2/((u+1)^2+1)
                    chunks = 1
                    cw = 512 // chunks
                    for cix in range(chunks):
                        cs = slice(cix * cw, (cix + 1) * cw)
                        psc = ps[:, cs]
                        u = fin.tile([128, cw], F32, tag=f"u{cix}", name="u")
                        nc.scalar.activation(u[:], psc, AF.Exp, bias=bias)
                        w2 = fin.tile([128, cw], F32, tag=f"w2{cix}", name="w2")
                        nc.scalar.activation(w2[:], u[:], AF.Square, bias=1.0)
                        dd = fin.tile([128, cw], F32, tag=f"dd{cix}", name="dd")
                        t = fin.tile([128, cw], F32, tag=f"t{cix}", name="t")
                        rr = fin.tile([128, cw], F32, tag=f"rr{cix}", name="rr")
                        if last:
                            nc.vector.tensor_scalar_add(dd[:], w2[:], 1.0)
                            nc.vector.reciprocal(rr[:], dd[:])
                            nc.scalar.activation(t[:], rr[:], AF.Identity,
                                                 bias=1.0, scale=-2.0)
                        else:
                            nc.gpsimd.tensor_scalar_add(dd[:], w2[:], 1.0)
                            nc.vector.reciprocal(rr[:], dd[:])
                            nc.gpsimd.tensor_scalar(t[:], rr[:], -2.0, 1.0,
                                                    ALU.mult, ALU.add)
                        m = fin.tile([128, cw], F32, tag=f"m{cix}", name="m")
                        nc.vector.scalar_tensor_tensor(m[:], psc, bias, t[:],
                                                       ALU.add, ALU.mult)
                        o = fin.tile([128, cw], F32, tag=f"o{cix}", name="o")
                        rslice = rgbf[oh][:, g * 512 + cix * cw:
                                          g * 512 + (cix + 1) * cw]
                        if last:
                            nc.vector.tensor_add(o[:], m[:],
                                                 rslice.bitcast(F32))
                        else:
                            nc.gpsimd.tensor_add(o[:], m[:],
                                                 rslice.bitcast(F32))
                        nc.sync.dma_start(
                            out[oh * 128:(oh + 1) * 128,
                                g * 512 + cix * cw:g * 512 + (cix + 1) * cw],
                            o[:])

                deferq = []

                def drain(k):
                    for _ in range(k):
                        if deferq:
                            deferq.pop(0)()

                for ig in range(IG):
                    lastig = ig == IG - 1
                    inline2 = lastig    # last ig: single-pass z, t2/t3 in
                    pend = []           # a long-lived s-pool slot
                    pts = []
                    zps1 = []          # [t0, t1] psum tiles, lazy
                    zps2 = []          # [t2, t3]

                    def flush1(pair, zps1=zps1, zps2=zps2, lastig=inline2):
                        pt, pr = pair
                        if not zps1:
                            zps1.extend(zpool.tile([128, 257], F32, tag="z",
                                                   name=f"zp1_{t}")
                                        for t in range(2))
                        if lastig and not zps2:
                            # last ig: inline single-pass for t2/t3 too, in a
                            # long-lived s-pool slot (bank-aligned regions)
                            z2 = spool.tile([128, 2, 512], F32, tag="s",
                                            name="zp2sp")
                            zps2.extend([(z2[:, tt, 0:257],
                                          z2[:, tt, 256:257],
                                          z2[:, tt, 0:256])
                                         for tt in range(2)])
                        h, jj = pr // 8, pr % 8
                        rhs = vT2[h][:, jj, :, 0:257]
                        for t in range(2):
                            nc.tensor.matmul(
                                zps1[t][:], pt[:, :, t * 128:(t + 1) * 128],
                                rhs, perf_mode=DR,
                                start=(pr == 0), stop=(pr == PAIRS - 1))
                        if lastig:
                            for t in range(2, 4):
                                nc.tensor.matmul(
                                    zps2[t - 2][0],
                                    pt[:, :, t * 128:(t + 1) * 128],
                                    rhs, perf_mode=DR,
                                    start=(pr == 0), stop=(pr == PAIRS - 1))

                    for pr in range(PAIRS):
                        gp = ig * PAIRS + pr
                        for task in weave.get(gp, []):
                            task()
                        ps = spool.tile([128, 2, 512], F32, tag="s", name="sT")
                        for hh in range(2):
                            jt = 2 * pr + hh
                            jsl = slice((jt % 16) * 128, (jt % 16 + 1) * 128)
                            nc.tensor.matmul(ps[:, hh], ksh[jt // 16][:, :, jsl],
                                             qsg[ig][:], perf_mode=DR,
                                             start=True, stop=True)
                        if len(pend) > 2:
                            flush1(pend.pop(0))
                        drain(4)
                        pt = pexp.tile([128, 2, 512], FP8, tag="pt", name="pt")
                        dset = (DVE_EXP_IG0 if ig == 0 else
                                DVE_EXP_LAST if lastig else DVE_EXP)
                        if pr in dset:
                            nc.vector.tensor_scalar(pt[:].bitcast(I8), ps[:],
                                                    A_SCH, B_SCH,
                                                    ALU.mult, ALU.add)
                        else:
                            nc.scalar.activation(pt[:], ps[:], AF.Exp)
                        pend.append((pt, pr))
                        pts.append(pt)

                    # ---- boundary work, deferred into next ig's stream ----
                    def boundary(ig=ig, pend=list(pend), pts=list(pts),
                                 zps1=zps1, zps2=zps2):
                        last = ig == IG - 1
                        rinvs = [None] * 4
                        zns = [None] * 4

                        def ftails():
                            for pair in pend:
                                flush1(pair, zps1=zps1)

                        def zn_one(t, src_den, src_dat):
                            rinvs[t] = znorm.tile([128, 1], F32,
                                                  tag=f"ri{t}", name="ri")
                            nc.vector.reciprocal(rinvs[t][:], src_den)
                            zns[t] = znorm.tile([128, 256], BF16,
                                                tag=f"zn{t}", name="zn")
                            if last and t % 2 == 0:
                                nc.scalar.activation(zns[t][:], src_dat,
                                                     AF.Identity,
                                                     scale=rinvs[t][:, 0:1])
                            else:
                                nc.vector.tensor_scalar_mul(
                                    zns[t][:], src_dat, rinvs[t][:])

                        def norm01():
                            for t in range(2):
                                zn_one(t, zps1[t][:, 256:257],
                                       zps1[t][:, 0:256])

                        def pass2(t):
                            if last:
                                return   # t2/t3 accumulated inline
                            if not zps2:
                                for tt in range(2):
                                    zt = zpool.tile([128, 257], F32, tag="z",
                                                    name=f"zp2_{tt}")
                                    zps2.append((zt[:], zt[:, 256:257],
                                                 zt[:, 0:256]))
                            for pr2, pt2 in enumerate(pts):
                                h, jj = pr2 // 8, pr2 % 8
                                nc.tensor.matmul(
                                    zps2[t - 2][0],
                                    pt2[:, :, t * 128:(t + 1) * 128],
                                    vT2[h][:, jj, :, 0:257], perf_mode=DR,
                                    start=(pr2 == 0), stop=(pr2 == PAIRS - 1))

                        def norm23():
                            for t in range(2, 4):
                                zn_one(t, zps2[t - 2][1], zps2[t - 2][2])

                        def transp(tp2):
                            # i-tiles 2*tp2, 2*tp2+1 for both chunks
                            for ch in range(2):
                                tp = zpool.tile([128, 256], BF16, tag="z",
                                                name="tp")
                                for k in range(2):
                                    t = 2 * tp2 + k
                                    nc.tensor.transpose(
                                        tp[:, k * 128:(k + 1) * 128],
                                        zns[t][:, ch * 128:(ch + 1) * 128],
                                        ident[:])
                                dst = zsg[ch][ig][:,
                                                  tp2 * 256:(tp2 + 1) * 256]
                                if last and ch == 0:
                                    nc.scalar.copy(dst, tp[:])
                                else:
                                    nc.vector.tensor_copy(dst, tp[:])

                        ph5ps = [None, None]

                        def mms():
                            ph5ps[0] = phase5_mm(ig, 0)
                            ph5ps[1] = phase5_mm(ig, 1)

                        if last:
                            return [ftails,
                                    lambda: (norm01(), norm23()),
                                    lambda: transp(0), lambda: transp(1),
                                    mms,
                                    lambda: phase5_chain(ig, 0, ph5ps[0],
                                                         last),
                                    lambda: phase5_chain(ig, 1, ph5ps[1],
                                                         last)]
                        return [ftails, norm01,
                                lambda: transp(0),
                                lambda: pass2(2), lambda: pass2(3),
                                norm23,
                                lambda: transp(1),
                                mms,
                                lambda: phase5_chain(ig, 0, ph5ps[0], last),
                                lambda: phase5_chain(ig, 1, ph5ps[1], last)]

                    deferq.extend(boundary())

                # drain the tail (last ig boundary work)
                while deferq:
                    drain(1)

    nc.finalize()
    return nc


def _blockdiag_T(w, g0, g1):
    """lhsT chunk: [[w[g0].T, 0], [0, w[g1].T]] as [128, 128]."""
    m = np.zeros((128, 128), dtype=np.float64)
    m[:64, :64] = w[g0].T
    m[64:, 64:] = w[g1].T
    return m


def prep_inputs(rgb, ir, w_q, b_q, w_k, b_k, w_v, b_v, w4, b4,
                gamma, beta, rmean, rvar):
    """Host-side prep: fold scale/BN/b_v, pack block-diagonal weights."""
    f64 = np.float64
    w_q, b_q = f64(np.asarray(w_q)), f64(np.asarray(b_q))
    w_k = f64(np.asarray(w_k))
    w_v, b_v = f64(np.asarray(w_v)), f64(np.asarray(b_v))
    w4, b4 = f64(np.asarray(w4)), f64(np.asarray(b4))
    gamma, beta = f64(np.asarray(gamma)), f64(np.asarray(beta))
    rmean, rvar = f64(np.asarray(rmean)), f64(np.asarray(rvar))

    inv = gamma / np.sqrt(rvar + 1e-5)
    w4f = w4 * inv[:, None]                      # BN folded into w4
    b4f = b4 * inv + beta - rmean * inv + w4f @ b_v   # b_v folded

    f32 = np.float32
    bf16 = ml_dtypes.bfloat16
    hs = np.sqrt(SCALE)  # split attention scale between q and k for fp8 range
    wq_bd = np.concatenate([_blockdiag_T(w_q * hs, 0, 1),
                            _blockdiag_T(w_q * hs, 2, 3)], axis=1).astype(f32)
    wk_bd = np.concatenate([_blockdiag_T(w_k * hs, 0, 1),
                            _blockdiag_T(w_k * hs, 2, 3)], axis=1).astype(f32)
    wv_r = np.zeros((128, 512), dtype=np.float64)
    wv_r[:, 0:128] = _blockdiag_T(w_v, 0, 1)
    wv_r[:, 384:512] = _blockdiag_T(w_v, 2, 3)
    wv_r = wv_r.astype(f32)
    w4t = np.zeros((128, 512), dtype=np.float64)
    for ch in range(2):
        for oh in range(2):
            w4t[:, ch * 256 + oh * 128:ch * 256 + (oh + 1) * 128] = \
                w4f[oh * 128:(oh + 1) * 128, ch * 128:(ch + 1) * 128].T
    w4t = w4t.astype(bf16)

    def cols(v):
        return np.stack([v[:128], v[128:]], axis=1).astype(np.float32)

    bq_c = cols(b_q * hs)
    bk_c = np.zeros((128, 2), dtype=np.float32)  # k bias cancels in softmax
    b4_c = cols(b4f)

    rgb_f = np.ascontiguousarray(np.asarray(rgb), dtype=np.float32)
    ir_f = np.ascontiguousarray(np.asarray(ir), dtype=np.float32)

    weights = dict(wq_bd=wq_bd, wk_bd=wk_bd, wv_r=wv_r, w4t=w4t,
                   bq=bq_c, bk=bk_c, b4=b4_c)
    in_maps = []
    for core in range(NCORES):
        b, half = divmod(core, 2)
        x_rgb = np.ascontiguousarray(
            rgb_f[b].reshape(C, N)[:, half * NH:(half + 1) * NH])
        x_ir = np.ascontiguousarray(ir_f[b].reshape(C, N))
        in_maps.append(dict(x_rgb=x_rgb, x_ir=x_ir, **weights))
    return in_maps


_PROGRAM = None


def _get_program():
    global _PROGRAM
    if _PROGRAM is None:
        _PROGRAM = build_program()
    return _PROGRAM


def run(inputs, trace=False, **kw):
    """Run on 8 cores; returns (full_output, BassKernelResults)."""
    nc = _get_program()
    in_maps = prep_inputs(**inputs)
    res = run_bass_kernel_spmd(nc, in_maps, list(range(NCORES)),
                               trace=trace, **kw)
    full = np.zeros((BS, C, H, W), dtype=np.float32)
    for core in range(NCORES):
        b, half = divmod(core, 2)
        full[b].reshape(C, N)[:, half * NH:(half + 1) * NH] = \
            res.results[core]["out"]
    return full, res


def kernel(**inputs) -> np.ndarray:
    out, _ = run(inputs)
    return out


# revision 9
# speedup vs baseline: 1.0381x; 1.0023x over previous
"""Trainium2 Bass kernel v2 for nn_CMF_Block (cross-modal fusion block).

Reference computation (per batch b):
    q = gconv1x1(rgb, w_q, b_q)   # [c, n]   c=256, n=h*w=4096, groups=4
    k = gconv1x1(ir,  w_k, b_k)
    v = gconv1x1(ir,  w_v, b_v)
    attn = softmax(q^T k * c^-0.5, axis=-1)      # [n, n]
    z = v @ attn^T                                # [c, n]
    y = w4 @ z + b4 ; y = BN(y) ; out = rgb + mish(y)

Sharding: 8 cores = 4 batches x 2 query-halves. Each core gets the full
ir slab [256, 4096] plus its rgb query-half [256, 2048] and produces the
matching disjoint output slice [256, 2048]. No collectives.

v2 design vs baseline (129.2us):
  - z matmul (P@V) in fp8e4 DoubleRow (0.5 cyc/col); P written as fp8 by
    the exp stage, vT staged fp8 with 272-stride r-padding (16B dual-fp8
    alignment). Runs as 2 passes of 2 i-tiles so zps needs only 2 psum
    banks, freeing 6 banks for a triple-buffered score pool.
  - softmax exp split across ACT (AF.Exp) and DVE (Schraudolph exp:
    int8(s*8/ln2 + 56) bitcast fp8e4; ~1.5e-4 end-to-end).
  - k bias dropped (constant per query in scores -> cancels in softmax).
  - mish phase mostly on the idle Pool engine (SBUF-only chain).
  - gconv psum shares the score pool slots, woven into the pair stream.
  - i-group boundary work (znorm, pass-2 z, transposes, phase5) is
    deferred and drained into the next i-group's pair stream so no engine
    queue blocks at boundaries; the last group uses a latency-optimized
    ACT/DVE-only phase5 in 256-col chunks.
  - prologue DMAs issue from 4 queues in parallel, critical chunks first.
"""

import sys

sys.path.insert(0, "/opt/trn_rl_repo")

import numpy as np
import ml_dtypes

import concourse.bass as bass
import concourse.tile as tile
from concourse import bacc
from concourse import mybir
from concourse.bass_utils import run_bass_kernel_spmd
from concourse.masks import make_identity

F32 = mybir.dt.float32
F32R = mybir.dt.float32r
BF16 = mybir.dt.bfloat16
FP8 = mybir.dt.float8e4
I8 = mybir.dt.int8
AF = mybir.ActivationFunctionType
DR = mybir.MatmulPerfMode.DoubleRow
ALU = mybir.AluOpType

BS, C, H, W = 4, 256, 64, 64
N = H * W              # 4096
G, CG = 4, 64
NH = N // 2            # 2048 query positions per core
NCORES = 8
SCALE = C ** -0.5      # 1/16

JT = N // 128          # 32 key tiles
IG = 4                 # i-groups of 512 queries
PAIRS = JT // 2        # 16 j-tile pairs per i-group

A_SCH = float(8.0 / np.log(2.0))   # fp8e4 Schraudolph scale
B_SCH = 56.0                       # fp8e4 exponent bias * 8

# exp engine split: pair indices (mod 16) sent to DVE-Schraudolph.
# Keep the first pairs of each ig on ACT so deferred boundary work on DVE
# isn't stuck behind an exp.
DVE_EXP = {4, 6, 8, 10, 12, 15}
DVE_EXP_LAST = {4, 6, 8, 10, 12, 15}
DVE_EXP_IG0 = {1, 2, 3, 4, 6, 8, 10, 12, 15}


def build_program():
    nc = bacc.Bacc("TRN2", target_bir_lowering=False, debug=False,
                   enable_asserts=False)

    x_rgb = nc.dram_tensor("x_rgb", [C, NH], F32R, kind="ExternalInput").ap()
    x_ir = nc.dram_tensor("x_ir", [C, N], F32R, kind="ExternalInput").ap()
    wq_bd = nc.dram_tensor("wq_bd", [128, 256], F32R, kind="ExternalInput").ap()
    wk_bd = nc.dram_tensor("wk_bd", [128, 256], F32R, kind="ExternalInput").ap()
    wv_r = nc.dram_tensor("wv_r", [128, 512], F32R, kind="ExternalInput").ap()
    w4t = nc.dram_tensor("w4t", [128, 512], BF16, kind="ExternalInput").ap()
    bq = nc.dram_tensor("bq", [128, 2], F32, kind="ExternalInput").ap()
    bk = nc.dram_tensor("bk", [128, 2], F32, kind="ExternalInput").ap()
    b4 = nc.dram_tensor("b4", [128, 2], F32, kind="ExternalInput").ap()
    out = nc.dram_tensor("out", [C, NH], F32, kind="ExternalOutput").ap()

    with tile.TileContext(nc) as tc:
        with tc.tile_pool(name="persist", bufs=1) as persist:
            qsg = [persist.tile([128, 2, 512], FP8, tag=f"qsg{g}",
                                name=f"qsg{g}") for g in range(IG)]
            ksh = [persist.tile([128, 2, 2048], FP8, tag=f"ksh{h}",
                                name=f"ksh{h}") for h in range(2)]
            # vT2 packs j-tile pairs for DoubleRow: [p, jj, r, c] =
            # v[(16h + 2jj + r)*128 + p, c]; col 256 = ones (softmax denom),
            # cols 257:272 pad for the 16B-aligned r-stride.
            vT2 = [persist.tile([128, 8, 2, 272], FP8, tag=f"vT2{h}",
                                name=f"vT2{h}") for h in range(2)]
            zsg = [[persist.tile([128, 512], BF16, tag=f"zsg{ch}_{g}",
                                 name=f"zsg{ch}_{g}") for g in range(IG)]
                   for ch in range(2)]
            rgbf = [persist.tile([128, NH], F32R, tag=f"rgbf{ch}",
                                 name=f"rgbf{ch}") for ch in range(2)]
            irf = [[persist.tile([128, 2048], F32R, tag=f"irf{ch}_{h}",
                                 name=f"irf{ch}_{h}") for h in range(2)]
                   for ch in range(2)]
            wq_sb = persist.tile([128, 2, 128], F32R, tag="wq_sb", name="wq_sb")
            wk_sb = persist.tile([128, 2, 128], F32R, tag="wk_sb", name="wk_sb")
            wv_sb = persist.tile([128, 2, 256], F32R, tag="wv_sb", name="wv_sb")
            w4_sb = persist.tile([128, 2, 2, 128], BF16, tag="w4_sb", name="w4_sb")
            bq_sb = persist.tile([128, 2], F32, tag="bq_sb", name="bq_sb")
            b4_sb = persist.tile([128, 2], F32, tag="b4_sb", name="b4_sb")
            ident = persist.tile([128, 128], BF16, tag="ident", name="ident")

            # ---- prologue DMAs: critical chunks first (HWDGE serializes
            # issues globally, so minimize count and front-load the head) ----
            nc.sync.dma_start(wk_sb[:], wk_bd)
            nc.sync.dma_start(irf[0][0][:, 0:1024], x_ir[0:128, 0:1024])
            nc.sync.dma_start(irf[1][0][:, 0:1024], x_ir[128:256, 0:1024])
            nc.sync.dma_start(wq_sb[:], wq_bd)
            nc.sync.dma_start(bq_sb[:], bq)
            nc.sync.dma_start(rgbf[0][:, 0:512], x_rgb[0:128, 0:512])
            nc.sync.dma_start(rgbf[1][:, 0:512], x_rgb[128:256, 0:512])
            nc.sync.dma_start(wv_sb[:], wv_r)
            nc.sync.dma_start(irf[0][0][:, 1024:2048], x_ir[0:128, 1024:2048])
            nc.sync.dma_start(irf[1][0][:, 1024:2048], x_ir[128:256, 1024:2048])
            nc.sync.dma_start(irf[0][1][:], x_ir[0:128, 2048:4096])
            nc.sync.dma_start(irf[1][1][:], x_ir[128:256, 2048:4096])
            nc.sync.dma_start(rgbf[0][:, 512:2048], x_rgb[0:128, 512:2048])
            nc.sync.dma_start(rgbf[1][:, 512:2048], x_rgb[128:256, 512:2048])
            nc.sync.dma_start(w4_sb[:], w4t)
            nc.sync.dma_start(b4_sb[:], b4)
            make_identity(nc, ident[:])
            for h in range(2):
                nc.gpsimd.memset(vT2[h][:, :, :, 256:257], 1.0)

            with (
                tc.tile_pool(name="spool", bufs=3, space="PSUM") as spool,
                tc.tile_pool(name="zpool", bufs=2, space="PSUM") as zpool,
                tc.tile_pool(name="pexp", bufs=20) as pexp,
                tc.tile_pool(name="znorm", bufs=8) as znorm,
                tc.tile_pool(name="fin", bufs=3) as fin,
            ):
                # ---- woven gconv slot tasks (share s-pool with pairs) ----
                mv_alt = [0]

                def mover(dst, src, bias=None):
                    mv_alt[0] ^= 1
                    if bias is None:
                        if mv_alt[0]:
                            nc.vector.tensor_copy(dst, src)
                        else:
                            nc.scalar.copy(dst, src)
                    else:
                        if mv_alt[0]:
                            nc.vector.tensor_scalar_add(dst, src, bias)
                        else:
                            nc.scalar.activation(dst, src, AF.Identity,
                                                 bias=bias)

                def kslot(h, ch, half, fast=False):
                    ps = spool.tile([128, 1024], F32, tag="s", name="kps")
                    for q4 in range(2):
                        csl = slice(half * 1024 + q4 * 512,
                                    half * 1024 + (q4 + 1) * 512)
                        nc.tensor.matmul(ps[:, q4 * 512:(q4 + 1) * 512],
                                         wk_sb[:, ch], irf[ch][h][:, csl],
                                         start=True, stop=True)
                        if fast:
                            mover(ksh[h][:, ch, csl],
                                  ps[:, q4 * 512:(q4 + 1) * 512])
                    if not fast:
                        csl = slice(half * 1024, (half + 1) * 1024)
                        mover(ksh[h][:, ch, csl], ps[:])

                def qslot(g):
                    gsl = slice(g * 512, (g + 1) * 512)
                    ps = spool.tile([128, 1024], F32, tag="s", name="qps")
                    for ch in range(2):
                        nc.tensor.matmul(ps[:, ch * 512:(ch + 1) * 512],
                                         wq_sb[:, ch], rgbf[ch][:, gsl],
                                         start=True, stop=True)
                    nc.vector.tensor_scalar_add(
                        qsg[g][:, 0, :], ps[:, 0:512], bq_sb[:, 0:1])
                    nc.vector.tensor_scalar_add(
                        qsg[g][:, 1, :], ps[:, 512:1024], bq_sb[:, 1:2])

                def vslot(h, q):
                    # 4 j-tiles (j = 4q .. 4q+3 within half h) -> vT2
                    ps = spool.tile([128, 1024], F32, tag="s", name="vps")
                    for jl in range(4):
                        j = 4 * q + jl
                        jsl = slice(j * 128, (j + 1) * 128)
                        psl = slice(jl * 256, (jl + 1) * 256)
                        for ch in range(2):
                            nc.tensor.matmul(ps[:, psl], irf[ch][h][:, jsl],
                                             wv_sb[:, ch],
                                             start=(ch == 0), stop=(ch == 1))
                    dst = vT2[h][:, 2 * q:2 * q + 2, :, 0:256]
                    mover(dst, ps[:])

                weave = {
                    0: [lambda: kslot(0, 0, 0), lambda: kslot(0, 1, 0),
                        lambda: qslot(0)],
                    1: [lambda: vslot(0, 0)],
                    2: [lambda: vslot(0, 1)],
                    3: [lambda: kslot(0, 0, 1)],
                    4: [lambda: kslot(0, 1, 1)],
                    5: [lambda: vslot(0, 2), lambda: kslot(1, 0, 0)],
                    6: [lambda: vslot(0, 3), lambda: kslot(1, 1, 0)],
                    7: [lambda: kslot(1, 0, 1)],
                    8: [lambda: kslot(1, 1, 1)],
                    9: [lambda: vslot(1, 0)],
                    10: [lambda: vslot(1, 1)],
                    11: [lambda: vslot(1, 2)],
                    12: [lambda: vslot(1, 3)],
                    13: [lambda: qslot(1)],
                    18: [lambda: qslot(2)],
                    34: [lambda: qslot(3)],
                }

                def phase5_mm(g, oh):
                    ps = zpool.tile([128, 512], F32, tag="z", name="yps")
                    for ch in range(2):
                        nc.tensor.matmul(ps[:], w4_sb[:, ch, oh],
                                         zsg[ch][g][:],
                                         start=(ch == 0), stop=(ch == 1))
                    return ps

                def phase5_chain(g, oh, ps, last):
                    bias = b4_sb[:, oh:oh + 1]
                    # mish(y) = y*tanh(softplus(y)); with u = e^y:
                    # tanh(softplus(y)) = 1 - 2/((u+1)^2+1)
                    chunks = 1
                    cw = 512 // chunks
                    for cix in range(chunks):
                        cs = slice(cix * cw, (cix + 1) * cw)
                        psc = ps[:, cs]
                        u = fin.tile([128, cw], F32, tag=f"u{cix}", name="u")
                        nc.scalar.activation(u[:], psc, AF.Exp, bias=bias)
                        w2 = fin.tile([128, cw], F32, tag=f"w2{cix}", name="w2")
                        nc.scalar.activation(w2[:], u[:], AF.Square, bias=1.0)
                        dd = fin.tile([128, cw], F32, tag=f"dd{cix}", name="dd")
                        t = fin.tile([128, cw], F32, tag=f"t{cix}", name="t")
                        rr = fin.tile([128, cw], F32, tag=f"rr{cix}", name="rr")
                        if last:
                            nc.vector.tensor_scalar_add(dd[:], w2[:], 1.0)
                            nc.vector.reciprocal(rr[:], dd[:])
                            nc.scalar.activation(t[:], rr[:], AF.Identity,
                                                 bias=1.0, scale=-2.0)
                        else:
                            nc.gpsimd.tensor_scalar_add(dd[:], w2[:], 1.0)
                            nc.vector.reciprocal(rr[:], dd[:])
                            nc.gpsimd.tensor_scalar(t[:], rr[:], -2.0, 1.0,
                                                    ALU.mult, ALU.add)
                        m = fin.tile([128, cw], F32, tag=f"m{cix}", name="m")
                        nc.vector.scalar_tensor_tensor(m[:], psc, bias, t[:],
                                                       ALU.add, ALU.mult)
                        o = fin.tile([128, cw], F32, tag=f"o{cix}", name="o")
                        rslice = rgbf[oh][:, g * 512 + cix * cw:
                                          g * 512 + (cix + 1) * cw]
                        if last:
                            nc.vector.tensor_add(o[:], m[:],
                                                 rslice.bitcast(F32))
                        else:
                            nc.gpsimd.tensor_add(o[:], m[:],
                                                 rslice.bitcast(F32))
                        nc.sync.dma_start(
                            out[oh * 128:(oh + 1) * 128,
                                g * 512 + cix * cw:g * 512 + (cix + 1) * cw],
                            o[:])

                deferq = []

                def drain(k):
                    for _ in range(k):
                        if deferq:
                            deferq.pop(0)()

                for ig in range(IG):
                    lastig = ig == IG - 1
                    inline2 = lastig    # last ig: single-pass z, t2/t3 in
                    pend = []           # a long-lived s-pool slot
                    pts = []
                    zps1 = []          # [t0, t1] psum tiles, lazy
                    zps2 = []          # [t2, t3]

                    def flush1(pair, zps1=zps1, zps2=zps2, lastig=inline2):
                        pt, pr = pair
                        if not zps1:
                            zps1.extend(zpool.tile([128, 257], F32, tag="z",
                                                   name=f"zp1_{t}")
                                        for t in range(2))
                        if lastig and not zps2:
                            # last ig: inline single-pass for t2/t3 too, in a
                            # long-lived s-pool slot (bank-aligned regions)
                            z2 = spool.tile([128, 2, 512], F32, tag="s",
                                            name="zp2sp")
                            zps2.extend([(z2[:, tt, 0:257],
                                          z2[:, tt, 256:257],
                                          z2[:, tt, 0:256])
                                         for tt in range(2)])
                        h, jj = pr // 8, pr % 8
                        rhs = vT2[h][:, jj, :, 0:257]
                        for t in range(2):
                            nc.tensor.matmul(
                                zps1[t][:], pt[:, :, t * 128:(t + 1) * 128],
                                rhs, perf_mode=DR,
                                start=(pr == 0), stop=(pr == PAIRS - 1))
                        if lastig:
                            for t in range(2, 4):
                                nc.tensor.matmul(
                                    zps2[t - 2][0],
                                    pt[:, :, t * 128:(t + 1) * 128],
                                    rhs, perf_mode=DR,
                                    start=(pr == 0), stop=(pr == PAIRS - 1))

                    for pr in range(PAIRS):
                        gp = ig * PAIRS + pr
                        for task in weave.get(gp, []):
                            task()
                        ps = spool.tile([128, 2, 512], F32, tag="s", name="sT")
                        for hh in range(2):
                            jt = 2 * pr + hh
                            jsl = slice((jt % 16) * 128, (jt % 16 + 1) * 128)
                            nc.tensor.matmul(ps[:, hh], ksh[jt // 16][:, :, jsl],
                                             qsg[ig][:], perf_mode=DR,
                                             start=True, stop=True)
                        if len(pend) > 2:
                            flush1(pend.pop(0))
                        drain(4)
                        pt = pexp.tile([128, 2, 512], FP8, tag="pt", name="pt")
                        dset = (DVE_EXP_IG0 if ig == 0 else
                                DVE_EXP_LAST if lastig else DVE_EXP)
                        if pr in dset:
                            nc.vector.tensor_scalar(pt[:].bitcast(I8), ps[:],
                                                    A_SCH, B_SCH,
                                                    ALU.mult, ALU.add)
                        else:
                            nc.scalar.activation(pt[:], ps[:], AF.Exp)
                        pend.append((pt, pr))
                        pts.append(pt)

                    # ---- boundary work, deferred into next ig's stream ----
                    def boundary(ig=ig, pend=list(pend), pts=list(pts),
                                 zps1=zps1, zps2=zps2):
                        last = ig == IG - 1
                        rinvs = [None] * 4
                        zns = [None] * 4

                        def ftails():
                            for pair in pend:
                                flush1(pair, zps1=zps1)

                        def zn_one(t, src_den, src_dat):
                            rinvs[t] = znorm.tile([128, 1], F32,
                                                  tag=f"ri{t}", name="ri")
                            nc.vector.reciprocal(rinvs[t][:], src_den)
                            zns[t] = znorm.tile([128, 256], BF16,
                                                tag=f"zn{t}", name="zn")
                            if last and t % 2 == 0:
                                nc.scalar.activation(zns[t][:], src_dat,
                                                     AF.Identity,
                                                     scale=rinvs[t][:, 0:1])
                            else:
                                nc.vector.tensor_scalar_mul(
                                    zns[t][:], src_dat, rinvs[t][:])

                        def norm01():
                            for t in range(2):
                                zn_one(t, zps1[t][:, 256:257],
                                       zps1[t][:, 0:256])

                        def pass2(t):
                            if last:
                                return   # t2/t3 accumulated inline
                            if not zps2:
                                for tt in range(2):
                                    zt = zpool.tile([128, 257], F32, tag="z",
                                                    name=f"zp2_{tt}")
                                    zps2.append((zt[:], zt[:, 256:257],
                                                 zt[:, 0:256]))
                            for pr2, pt2 in enumerate(pts):
                                h, jj = pr2 // 8, pr2 % 8
                                nc.tensor.matmul(
                                    zps2[t - 2][0],
                                    pt2[:, :, t * 128:(t + 1) * 128],
                                    vT2[h][:, jj, :, 0:257], perf_mode=DR,
                                    start=(pr2 == 0), stop=(pr2 == PAIRS - 1))

                        def norm23():
                            for t in range(2, 4):
                                zn_one(t, zps2[t - 2][1], zps2[t - 2][2])

                        def transp(tp2):
                            # i-tiles 2*tp2, 2*tp2+1 for both chunks
                            for ch in range(2):
                                tp = zpool.tile([128, 256], BF16, tag="z",
                                                name="tp")
                                for k in range(2):
                                    t = 2 * tp2 + k
                                    nc.tensor.transpose(
                                        tp[:, k * 128:(k + 1) * 128],
                                        zns[t][:, ch * 128:(ch + 1) * 128],
                                        ident[:])
                                dst = zsg[ch][ig][:,
                                                  tp2 * 256:(tp2 + 1) * 256]
                                if last and ch == 0:
                                    nc.scalar.copy(dst, tp[:])
                                else:
                                    nc.vector.tensor_copy(dst, tp[:])

                        ph5ps = [None, None]

                        def mms():
                            ph5ps[0] = phase5_mm(ig, 0)
                            ph5ps[1] = phase5_mm(ig, 1)

                        if last:
                            return [ftails,
                                    lambda: (norm01(), norm23()),
                                    lambda: transp(0), lambda: transp(1),
                                    mms,
                                    lambda: phase5_chain(ig, 0, ph5ps[0],
                                                         last),
                                    lambda: phase5_chain(ig, 1, ph5ps[1],
                                                         last)]
                        return [ftails, norm01,
                                lambda: transp(0),
                                lambda: pass2(2), lambda: pass2(3),
                                norm23,
                                lambda: transp(1),
                                mms,
                                lambda: phase5_chain(ig, 0, ph5ps[0], last),
                                lambda: phase5_chain(ig, 1, ph5ps[1], last)]

                    deferq.extend(boundary())

                # drain the tail (last ig boundary work)
                while deferq:
                    drain(1)

    nc.finalize()
    return nc


def _blockdiag_T(w, g0, g1):
    """lhsT chunk: [[w[g0].T, 0], [0, w[g1].T]] as [128, 128]."""
    m = np.zeros((128, 128), dtype=np.float64)
    m[:64, :64] = w[g0].T
    m[64:, 64:] = w[g1].T
    return m


def prep_inputs(rgb, ir, w_q, b_q, w_k, b_k, w_v, b_v, w4, b4,
                gamma, beta, rmean, rvar):
    """Host-side prep: fold scale/BN/b_v, pack block-diagonal weights."""
    f64 = np.float64
    w_q, b_q = f64(np.asarray(w_q)), f64(np.asarray(b_q))
    w_k = f64(np.asarray(w_k))
    w_v, b_v = f64(np.asarray(w_v)), f64(np.asarray(b_v))
    w4, b4 = f64(np.asarray(w4)), f64(np.asarray(b4))
    gamma, beta = f64(np.asarray(gamma)), f64(np.asarray(beta))
    rmean, rvar = f64(np.asarray(rmean)), f64(np.asarray(rvar))

    inv = gamma / np.sqrt(rvar + 1e-5)
    w4f = w4 * inv[:, None]                      # BN folded into w4
    b4f = b4 * inv + beta - rmean * inv + w4f @ b_v   # b_v folded

    f32 = np.float32
    bf16 = ml_dtypes.bfloat16
    hs = np.sqrt(SCALE)  # split attention scale between q and k for fp8 range
    wq_bd = np.concatenate([_blockdiag_T(w_q * hs, 0, 1),
                            _blockdiag_T(w_q * hs, 2, 3)], axis=1).astype(f32)
    wk_bd = np.concatenate([_blockdiag_T(w_k * hs, 0, 1),
                            _blockdiag_T(w_k * hs, 2, 3)], axis=1).astype(f32)
    wv_r = np.zeros((128, 512), dtype=np.float64)
    wv_r[:, 0:128] = _blockdiag_T(w_v, 0, 1)
    wv_r[:, 384:512] = _blockdiag_T(w_v, 2, 3)
    wv_r = wv_r.astype(f32)
    w4t = np.zeros((128, 512), dtype=np.float64)
    for ch in range(2):
        for oh in range(2):
            w4t[:, ch * 256 + oh * 128:ch * 256 + (oh + 1) * 128] = \
                w4f[oh * 128:(oh + 1) * 128, ch * 128:(ch + 1) * 128].T
    w4t = w4t.astype(bf16)

    def cols(v):
        return np.stack([v[:128], v[128:]], axis=1).astype(np.float32)

    bq_c = cols(b_q * hs)
    bk_c = np.zeros((128, 2), dtype=np.float32)  # k bias cancels in softmax
    b4_c = cols(b4f)

    rgb_f = np.ascontiguousarray(np.asarray(rgb), dtype=np.float32)
    ir_f = np.ascontiguousarray(np.asarray(ir), dtype=np.float32)

    weights = dict(wq_bd=wq_bd, wk_bd=wk_bd, wv_r=wv_r, w4t=w4t,
                   bq=bq_c, bk=bk_c, b4=b4_c)
    in_maps = []
    for core in range(NCORES):
        b, half = divmod(core, 2)
        x_rgb = np.ascontiguousarray(
            rgb_f[b].reshape(C, N)[:, half * NH:(half + 1) * NH])
        x_ir = np.ascontiguousarray(ir_f[b].reshape(C, N))
        in_maps.append(dict(x_rgb=x_rgb, x_ir=x_ir, **weights))
    return in_maps


_PROGRAM = None


def _get_program():
    global _PROGRAM
    if _PROGRAM is None:
        _PROGRAM = build_program()
    return _PROGRAM


def run(inputs, trace=False, **kw):
    """Run on 8 cores; returns (full_output, BassKernelResults)."""
    nc = _get_program()
    in_maps = prep_inputs(**inputs)
    res = run_bass_kernel_spmd(nc, in_maps, list(range(NCORES)),
                               trace=trace, **kw)
    full = np.zeros((BS, C, H, W), dtype=np.float32)
    for core in range(NCORES):
        b, half = divmod(core, 2)
        full[b].reshape(C, N)[:, half * NH:(half + 1) * NH] = \
            res.results[core]["out"]
    return full, res


def kernel(**inputs) -> np.ndarray:
    out, _ = run(inputs)
    return out
